# revision 1
# baseline (speedup 1.0000x reference)
"""Trainium2 Bass kernel for nn_AttentionFilter (B=2,C=128,H=256,W=510).

Sharding: 8 cores = 2 batches x 4 channel-groups of 32. Per core:
  1x1 conv (channel mix, M=32 matmul) -> DRAM spill -> per-channel:
  PE-transpose -> rfft_w (DFT matmul) -> fft_h (complex DFT matmul) ->
  filter mult (DVE) -> variance (bn_stats) -> freq attention matmul ->
  sigmoid (ACT, fused 1/sqrt(2pi var) row scale) -> ifft_i (transposed
  output) -> irfft_j -> residual add -> channel-LN partial stats.
  Cross-core AllReduce of LN stats within each batch group, then
  normalize. All matmuls run as float32r (full-rate fp32).
"""
import sys

sys.path.insert(0, "/opt/trn_rl_repo")

import numpy as np

import concourse.bass as bass
import concourse.mybir as mybir
import concourse.tile as tile
from concourse import bacc
from concourse.bass_utils import run_bass_kernel_spmd
from concourse.masks import make_identity

B, C, H, W = 2, 128, 256, 510
WF = 256
NCH = 32  # channels per core
N_CORES = 8
F32 = mybir.dt.float32
F32R = mybir.dt.float32r
AF = mybir.ActivationFunctionType


def _r(ap):
    return ap.bitcast(F32R)


def build_consts():
    Fw = np.fft.rfft(np.eye(W), axis=0, norm="ortho").T  # [W, WF]
    fw_pack = np.zeros((512, 512), np.float32)
    fw_pack[:W, :WF] = Fw.real
    fw_pack[:W, WF:] = Fw.imag
    DH = np.fft.fft(np.eye(H), axis=0, norm="ortho")  # [kh, h]
    dht_r = np.ascontiguousarray(DH.real.T, np.float32)  # [h, kh]
    dht_i = np.ascontiguousarray(DH.imag.T, np.float32)
    dht_ni = np.ascontiguousarray((-DH.imag).T, np.float32)
    IDH = np.fft.ifft(np.eye(H), axis=0, norm="ortho")  # [m, i]
    idht_r = np.ascontiguousarray(IDH.real.T, np.float32)  # [i, m]
    idht_i = np.ascontiguousarray(IDH.imag.T, np.float32)
    idht_ni = np.ascontiguousarray((-IDH.imag).T, np.float32)
    AR = np.fft.irfft(np.eye(WF), n=W, axis=0, norm="ortho")  # [n, j]
    AI = np.fft.irfft(1j * np.eye(WF), n=W, axis=0, norm="ortho")
    awr = np.ascontiguousarray(AR.T, np.float32)  # [j, n]
    awi = np.ascontiguousarray(AI.T, np.float32)
    return dict(fw=fw_pack, dht_r=dht_r, dht_i=dht_i, dht_ni=dht_ni,
                idht_r=idht_r, idht_i=idht_i, idht_ni=idht_ni,
                awr=awr, awi=awi)


def build_program():
    nc = bacc.Bacc("TRN2", target_bir_lowering=False, debug=False,
                   num_devices=N_CORES)

    def inp(name, shape, dt=F32):
        return nc.dram_tensor(name, list(shape), dt, kind="ExternalInput").ap()

    gb = inp("gb", (C, H, W), mybir.dt.float16)
    xb = inp("xb", (C, H, W), mybir.dt.float16)
    xres = inp("xres", (NCH, H, W))
    wgT = inp("wgT", (C, NCH), mybir.dt.float16)
    wxT = inp("wxT", (C, NCH), mybir.dt.float16)
    bg = inp("bg", (128, 1))
    bx = inp("bx", (128, 1))
    fw = inp("fw", (512, 512), F32R)
    dht_r = inp("dht_r", (H, H), F32R)
    dht_i = inp("dht_i", (H, H), F32R)
    dht_ni = inp("dht_ni", (H, H), F32R)
    idht_r = inp("idht_r", (H, H), F32R)
    idht_i = inp("idht_i", (H, H), F32R)
    idht_ni = inp("idht_ni", (H, H), F32R)
    awr = inp("awr", (WF, W), F32R)
    awi = inp("awi", (WF, W), F32R)
    fpg = inp("fpg", (NCH, 2, H, WF), mybir.dt.float16)
    fpx = inp("fpx", (NCH, 2, H, WF), mybir.dt.float16)
    gamma = inp("gamma", (1, NCH))
    beta = inp("beta", (1, NCH))
    out = nc.dram_tensor("out", [NCH, H, W], F32, kind="ExternalOutput").ap()
    dbg_zg = nc.dram_tensor("dbg_zg", [128, 2, 512], F32R,
                            kind="ExternalOutput").ap()
    dbg_att = nc.dram_tensor("dbg_att", [128, 2, 512], F32R,
                             kind="ExternalOutput").ap()
    dbg_rstd = nc.dram_tensor("dbg_rstd", [128, 2], F32,
                              kind="ExternalOutput").ap()

    with tile.TileContext(nc) as tc:
        with (
            tc.tile_pool(name="consts", bufs=1) as consts,
            tc.tile_pool(name="dram", bufs=1, space="DRAM") as dram,
        ):
            # ---- constants into SBUF
            c_fw = consts.tile([128, 4, 512], F32R)
            nc.sync.dma_start(c_fw, fw.rearrange("(wc p) n -> p wc n", p=128))
            def ld2(src):
                t = consts.tile([128, 2, H], F32R, tag=f"c_{src.name}")
                nc.sync.dma_start(t, src.rearrange("(hc p) m -> p hc m", p=128))
                return t
            c_dhtr, c_dhti, c_dhtni = ld2(dht_r), ld2(dht_i), ld2(dht_ni)
            c_idr, c_idi, c_idni = ld2(idht_r), ld2(idht_i), ld2(idht_ni)
            c_awr = consts.tile([128, 2, W], F32R, tag="c_awr")
            nc.sync.dma_start(c_awr, awr.rearrange("(jc p) n -> p jc n", p=128))
            c_awi = consts.tile([128, 2, W], F32R, tag="c_awi")
            nc.sync.dma_start(c_awi, awi.rearrange("(jc p) n -> p jc n", p=128))
            c_wgT = consts.tile([128, NCH], mybir.dt.float16, tag="c_wgT")
            nc.sync.dma_start(c_wgT, wgT)
            c_wxT = consts.tile([128, NCH], mybir.dt.float16, tag="c_wxT")
            nc.sync.dma_start(c_wxT, wxT)
            c_bg4 = consts.tile([128, 1], F32, tag="c_bg4")
            nc.sync.dma_start(c_bg4, bg)
            c_bx4 = consts.tile([128, 1], F32, tag="c_bx4")
            nc.sync.dma_start(c_bx4, bx)
            c_gamma = consts.tile([128, NCH], F32, tag="c_gamma")
            nc.sync.dma_start(c_gamma, gamma.to_broadcast([128, NCH]))
            c_beta = consts.tile([128, NCH], F32, tag="c_beta")
            nc.sync.dma_start(c_beta, beta.to_broadcast([128, NCH]))
            c_eps = consts.tile([128, 1], F32, tag="c_eps")
            nc.vector.memset(c_eps, 1e-6)
            c_ident = consts.tile([128, 128], mybir.dt.float16,
                                  tag="c_ident")
            make_identity(nc, c_ident)

            # ---- DRAM scratch
            ysp_g = dram.tile([NCH, H, W], mybir.dt.float16, tag="ysp_g")
            ysp_x = dram.tile([NCH, H, W], mybir.dt.float16, tag="ysp_x")
            r_sp = dram.tile([NCH, H, W], mybir.dt.float16, tag="r_sp")
            cc_in = dram.tile([128, 2 * 1020], F32, tag="cc_in")
            cc_out = dram.tile([128, 2 * 1020], F32, tag="cc_out")

            # ---- Phase A: 1x1 conv, spill y to DRAM
            HB = 32  # h-rows per block
            with (
                tc.tile_pool(name="pa_in", bufs=2) as pa_in,
                tc.tile_pool(name="pa_out", bufs=3) as pa_out,
                tc.tile_pool(name="pa_ps", bufs=4, space="PSUM") as pa_ps,
            ):
                for srct, wTt, biast, yspt in ((gb, c_wgT, c_bg4, ysp_g),
                                               (xb, c_wxT, c_bx4, ysp_x)):
                    for blk in range(H // HB):
                        h0 = blk * HB
                        rh = pa_in.tile([128, HB, W], mybir.dt.float16,
                                        tag="rh")
                        nc.sync.dma_start(rh, srct[:, h0:h0 + HB, :])
                        stag = pa_out.tile([128, HB // 4, W],
                                           mybir.dt.float16, tag="stag")
                        for i2 in range(HB // 4):
                            ps = pa_ps.tile([128, W], F32, tag="cps")
                            for j in range(4):
                                nc.tensor.matmul(
                                    ps[32 * j:32 * (j + 1), :], wTt,
                                    rh[:, i2 * 4 + j, :],
                                    start=True, stop=True,
                                    tile_position=(0, 32 * j))
                            nc.scalar.activation(stag[:, i2, :], ps,
                                                 AF.Identity, bias=biast)
                        for j in range(4):
                            nc.sync.dma_start(
                                yspt[:, h0 + j:h0 + HB:4, :],
                                stag[32 * j:32 * (j + 1), :, :])

            # ---- Phase B: per-channel frequency pipeline
            with (
                tc.tile_pool(name="pb_ld", bufs=3) as pb_ld,
                tc.tile_pool(name="pb_yt", bufs=3) as pb_yt,
                tc.tile_pool(name="pb_yw", bufs=3) as pb_yw,
                tc.tile_pool(name="pb_z", bufs=3) as pb_z,
                tc.tile_pool(name="pb_f", bufs=3) as pb_f,
                tc.tile_pool(name="pb_sm", bufs=3) as pb_sm,
                tc.tile_pool(name="pb_att", bufs=3) as pb_att,
                tc.tile_pool(name="pb_inv", bufs=3) as pb_inv,
                tc.tile_pool(name="pb_r", bufs=2) as pb_r,
                tc.tile_pool(name="pb_acc", bufs=1) as pb_acc,
                tc.tile_pool(name="pb_rl", bufs=3) as pb_rl,
                tc.tile_pool(name="pb_tps", bufs=1, space="PSUM") as pb_tps,
                tc.tile_pool(name="pb_ps", bufs=2, space="PSUM") as pb_ps,
                tc.tile_pool(name="pb_ps5", bufs=3, space="PSUM") as pb_ps5,
            ):
                S1 = pb_acc.tile([128, 2, W], F32, tag="S1")
                S2 = pb_acc.tile([128, 2, W], F32, tag="S2")
                nc.vector.memset(S1, 0.0)
                nc.vector.memset(S2, 0.0)

                for c in range(NCH):
                    z = {}   # (t, 'R'/'I') -> [128, 2(khc), 256]
                    for t, ysp, fPd in ((0, ysp_g, fpg), (1, ysp_x, fpx)):
                        yld = pb_ld.tile([128, 2, W], mybir.dt.float16,
                                         tag="yld")
                        nc.sync.dma_start(
                            yld, ysp[c].rearrange("(hc p) w -> p hc w", p=128))
                        ytT = pb_yt.tile([128, 4, H], F32R, tag="ytT")
                        for hc in range(2):
                            tp = pb_tps.tile([128, 4, 128], mybir.dt.float16,
                                             tag="tp")
                            for wc in range(4):
                                wl = 126 if wc == 3 else 128
                                nc.tensor.transpose(
                                    tp[:wl, wc, :],
                                    yld[:, hc, wc * 128:wc * 128 + wl], c_ident)
                            nc.scalar.copy(
                                out=ytT[:, :, hc * 128:(hc + 1) * 128],
                                in_=tp)
                        # B1: rfft_w -> yw [128(h), hc, kwR|kwI]
                        yw = pb_yw.tile([128, 2, 512], F32R, tag="yw")
                        for hc in range(2):
                            pw = pb_ps5.tile([128, 512], F32, tag="ps512")
                            for wc in range(4):
                                wl = 126 if wc == 3 else 128
                                nc.tensor.matmul(
                                    pw,
                                    _r(ytT[:wl, wc, hc * 128:(hc + 1) * 128]),
                                    _r(c_fw[:wl, wc, :]),
                                    start=(wc == 0), stop=(wc == 3))
                            nc.vector.tensor_copy(out=yw[:, hc, :], in_=pw)
                        # B2: fft_h (complex) + B3 filter
                        fP = pb_f.tile([128, 2, 2, WF], mybir.dt.float16,
                                       tag="fP")
                        nc.sync.dma_start(
                            fP, fPd[c].rearrange("ri (hc p) k -> p ri hc k",
                                                 p=128))
                        fR, fI = fP[:, 0], fP[:, 1]
                        zR = pb_z.tile([128, 2, WF], F32R, tag=f"zR{t}")
                        zI = pb_z.tile([128, 2, WF], F32R, tag=f"zI{t}")
                        for khc in range(2):
                            pfR = pb_ps.tile([128, 256], F32, tag="psA")
                            pfI = pb_ps.tile([128, 256], F32, tag="psB")
                            ksl = slice(khc * 128, (khc + 1) * 128)
                            for hc in range(2):
                                nc.tensor.matmul(
                                    pfR, _r(c_dhtr[:, hc, ksl]),
                                    _r(yw[:, hc, 0:256]),
                                    start=(hc == 0), stop=False)
                                nc.tensor.matmul(
                                    pfR, _r(c_dhtni[:, hc, ksl]),
                                    _r(yw[:, hc, 256:512]),
                                    start=False, stop=(hc == 1))
                                nc.tensor.matmul(
                                    pfI, _r(c_dhti[:, hc, ksl]),
                                    _r(yw[:, hc, 0:256]),
                                    start=(hc == 0), stop=False)
                                nc.tensor.matmul(
                                    pfI, _r(c_dhtr[:, hc, ksl]),
                                    _r(yw[:, hc, 256:512]),
                                    start=False, stop=(hc == 1))
                            t1 = pb_sm.tile([128, WF], F32, tag="t1")
                            t2 = pb_sm.tile([128, WF], F32, tag="t2")
                            t3 = pb_sm.tile([128, WF], F32, tag="t3")
                            t4 = pb_sm.tile([128, WF], F32, tag="t4")
                            nc.vector.tensor_mul(t1, pfR, fR[:, khc, :])
                            nc.vector.tensor_mul(t2, pfI, fI[:, khc, :])
                            nc.gpsimd.tensor_sub(zR[:, khc, :], t1, t2)
                            nc.vector.tensor_mul(t3, pfR, fI[:, khc, :])
                            nc.vector.tensor_mul(t4, pfI, fR[:, khc, :])
                            nc.gpsimd.tensor_add(zI[:, khc, :], t3, t4)
                        z[(t, "R")] = zR
                        z[(t, "I")] = zI

                    gR, gI = z[(0, "R")], z[(0, "I")]
                    xR, xI = z[(1, "R")], z[(1, "I")]
                    # neg imag of x for the scores real part
                    nxI = pb_z.tile([128, 2, WF], F32R, tag="nxI")
                    for khc in range(2):
                        nc.gpsimd.tensor_scalar_mul(
                            nxI[:, khc, :], xI[:, khc, :].bitcast(F32), -1.0)
                    # B4: variance over kw per kh row, rstd = 1/sqrt(2pi var)
                    rstd = pb_sm.tile([128, 2], F32, tag="rstd")
                    for khc in range(2):
                        st = pb_sm.tile([128, 2, 6], F32, tag="bst")
                        nc.vector.bn_stats(out=st[:, 0, :],
                                           in_=gR[:, khc, :].bitcast(F32))
                        nc.vector.bn_stats(out=st[:, 1, :],
                                           in_=gI[:, khc, :].bitcast(F32))
                        mvR = pb_sm.tile([128, 2], F32, tag="mvR")
                        mvI = pb_sm.tile([128, 2], F32, tag="mvI")
                        nc.vector.bn_aggr(out=mvR, in_=st[:, 0, :])
                        nc.vector.bn_aggr(out=mvI, in_=st[:, 1, :])
                        vs = pb_sm.tile([128, 1], F32, tag="vs")
                        nc.vector.tensor_add(vs, mvR[:, 1:2], mvI[:, 1:2])
                        # vs2 = 2*pi*var, explicitly (ACT scale is unreliable)
                        vs2 = pb_sm.tile([128, 1], F32, tag="vs2")
                        nc.vector.tensor_scalar_mul(vs2, vs,
                                                    float(2.0 * np.pi))
                        sd = pb_sm.tile([128, 1], F32, tag="sd")
                        nc.scalar.activation(sd, vs2, AF.Sqrt)
                        y0 = pb_sm.tile([128, 1], F32, tag="y0")
                        nc.vector.reciprocal(y0, sd)
                        # one Newton step: y1 = y0*(1.5 - 0.5*vs2*y0^2)
                        t_n = pb_sm.tile([128, 1], F32, tag="t_n")
                        nc.vector.tensor_mul(t_n, y0, y0)
                        nc.vector.tensor_mul(t_n, t_n, vs2)
                        nc.vector.tensor_scalar(
                            out=t_n, in0=t_n,
                            scalar1=-0.5, scalar2=1.5,
                            op0=mybir.AluOpType.mult,
                            op1=mybir.AluOpType.add)
                        nc.vector.tensor_mul(rstd[:, khc:khc + 1], y0, t_n)
                    if c == 0:
                        nc.sync.dma_start(dbg_zg[:, :, 0:256], gR)
                        nc.sync.dma_start(dbg_zg[:, :, 256:512], gI)
                        nc.sync.dma_start(dbg_rstd, rstd)
                    # B5 scores + B6 sigmoid with fused row scale
                    att = pb_att.tile([128, 2, 512], F32R, tag="att")
                    for ic in range(2):
                        pscR = pb_ps.tile([128, 256], F32, tag="psA")
                        pscI = pb_ps.tile([128, 256], F32, tag="psB")
                        isl = slice(ic * 128, (ic + 1) * 128)
                        for khc in range(2):
                            nc.tensor.matmul(pscR,
                                             _r(gR[:, khc, isl]),
                                             _r(xR[:, khc, :]),
                                             start=(khc == 0), stop=False)
                            nc.tensor.matmul(pscR,
                                             _r(gI[:, khc, isl]),
                                             _r(nxI[:, khc, :]),
                                             start=False, stop=(khc == 1))
                            nc.tensor.matmul(pscI,
                                             _r(gR[:, khc, isl]),
                                             _r(xI[:, khc, :]),
                                             start=(khc == 0), stop=False)
                            nc.tensor.matmul(pscI,
                                             _r(gI[:, khc, isl]),
                                             _r(xR[:, khc, :]),
                                             start=False, stop=(khc == 1))
                        nc.scalar.activation(att[:, ic, 0:256], pscR,
                                             AF.Sigmoid,
                                             scale=rstd[:, ic:ic + 1])
                        nc.scalar.activation(att[:, ic, 256:512], pscI,
                                             AF.Sigmoid,
                                             scale=rstd[:, ic:ic + 1])
                    if c == 0:
                        nc.sync.dma_start(dbg_att, att)
                    # B7: ifft over i, output transposed [j, mR|mI]
                    inv = pb_inv.tile([128, 2, 512], F32R, tag="inv")
                    for jc in range(2):
                        pvR = pb_ps.tile([128, 256], F32, tag="psA")
                        pvI = pb_ps.tile([128, 256], F32, tag="psB")
                        jsl = slice(jc * 128, (jc + 1) * 128)
                        jsl2 = slice(256 + jc * 128, 256 + (jc + 1) * 128)
                        for ic in range(2):
                            nc.tensor.matmul(pvR,
                                             _r(att[:, ic, jsl]),
                                             _r(c_idr[:, ic, :]),
                                             start=(ic == 0), stop=False)
                            nc.tensor.matmul(pvR,
                                             _r(att[:, ic, jsl2]),
                                             _r(c_idni[:, ic, :]),
                                             start=False, stop=(ic == 1))
                            nc.tensor.matmul(pvI,
                                             _r(att[:, ic, jsl]),
                                             _r(c_idi[:, ic, :]),
                                             start=(ic == 0), stop=False)
                            nc.tensor.matmul(pvI,
                                             _r(att[:, ic, jsl2]),
                                             _r(c_idr[:, ic, :]),
                                             start=False, stop=(ic == 1))
                        nc.scalar.copy(out=inv[:, jc, 0:256], in_=pvR)
                        nc.scalar.copy(out=inv[:, jc, 256:512], in_=pvI)
                    # B8 irfft over j + B9 residual & stats
                    rc2 = pb_r.tile([128, 2, W], mybir.dt.float16,
                                    tag="rc2")
                    xc2 = pb_r.tile([128, 2, W], F32, tag="xc2")
                    nc.sync.dma_start(
                        xc2, xres[c].rearrange("(hc p) w -> p hc w", p=128))
                    for mc in range(2):
                        pr = pb_ps5.tile([128, W], F32, tag="ps512")
                        msl = slice(mc * 128, (mc + 1) * 128)
                        msl2 = slice(256 + mc * 128, 256 + (mc + 1) * 128)
                        for jc in range(2):
                            nc.tensor.matmul(pr, _r(inv[:, jc, msl]),
                                             _r(c_awr[:, jc, :]),
                                             start=(jc == 0), stop=False)
                            nc.tensor.matmul(pr, _r(inv[:, jc, msl2]),
                                             _r(c_awi[:, jc, :]),
                                             start=False, stop=(jc == 1))
                        rc = rc2[:, mc, :]
                        nc.vector.tensor_add(rc, pr, xc2[:, mc, :])
                        nc.vector.tensor_add(S1[:, mc, :], S1[:, mc, :], rc)
                        sq = pb_r.tile([128, W], F32, tag="sq")
                        nc.gpsimd.tensor_mul(sq, rc, rc)
                        nc.gpsimd.tensor_add(S2[:, mc, :], S2[:, mc, :], sq)
                    nc.sync.dma_start(
                        r_sp[c].rearrange("(hc p) w -> p hc w", p=128), rc2)

                # ---- Phase C: LN stats AllReduce + normalize
                nc.sync.dma_start(cc_in[:, 0:1020],
                                  S1.rearrange("p a b -> p (a b)"))
                nc.sync.dma_start(cc_in[:, 1020:2040],
                                  S2.rearrange("p a b -> p (a b)"))
                nc.gpsimd.collective_compute(
                    "AllReduce", mybir.AluOpType.add,
                    replica_groups=[[0, 1, 2, 3], [4, 5, 6, 7]],
                    ins=[cc_in.opt()], outs=[cc_out.opt()])
                mu = pb_acc.tile([128, 1020], F32, tag="mu")
                var = pb_acc.tile([128, 1020], F32, tag="var")
                nc.sync.dma_start(mu, cc_out[:, 0:1020])
                nc.sync.dma_start(var, cc_out[:, 1020:2040])
                nc.scalar.mul(out=mu, in_=mu, mul=1.0 / C)
                nc.scalar.mul(out=var, in_=var, mul=1.0 / C)
                ta = pb_acc.tile([128, 1020], F32, tag="ta")
                nc.vector.tensor_mul(ta, mu, mu)
                nc.vector.tensor_sub(var, var, ta)  # var = E[r^2]-mu^2
                nc.vector.tensor_scalar_add(var, var, 1e-6)
                nc.scalar.activation(ta, var, AF.Sqrt)
                y0l = pb_acc.tile([128, 1020], F32, tag="y0l")
                nc.vector.reciprocal(y0l, ta)
                nc.vector.tensor_mul(ta, y0l, y0l)
                nc.vector.tensor_mul(ta, ta, var)
                nc.vector.tensor_scalar(
                    out=ta, in0=ta, scalar1=-0.5, scalar2=1.5,
                    op0=mybir.AluOpType.mult, op1=mybir.AluOpType.add)
                rstd_ln = pb_acc.tile([128, 1020], F32, tag="rstd_ln")
                nc.vector.tensor_mul(rstd_ln, y0l, ta)
                for c0 in range(0, NCH, 4):
                    for mc in range(2):
                        msl = slice(mc * 128, (mc + 1) * 128)
                        wsl = slice(mc * W, (mc + 1) * W)
                        rl4 = pb_rl.tile([128, 4, W], mybir.dt.float16,
                                         tag="rl4")
                        nc.sync.dma_start(
                            rl4, r_sp[c0:c0 + 4, msl, :].rearrange(
                                "c p w -> p c w"))
                        ob4 = pb_r.tile([128, 4, W], F32, tag="ob4")
                        for ci in range(4):
                            tt = pb_r.tile([128, W], F32, tag="tt")
                            nc.gpsimd.tensor_sub(tt, rl4[:, ci, :], mu[:, wsl])
                            nc.vector.tensor_mul(tt, tt, rstd_ln[:, wsl])
                            nc.vector.tensor_scalar(
                                out=ob4[:, ci, :], in0=tt,
                                scalar1=c_gamma[:, c0 + ci:c0 + ci + 1],
                                scalar2=c_beta[:, c0 + ci:c0 + ci + 1],
                                op0=mybir.AluOpType.mult,
                                op1=mybir.AluOpType.add)
                        nc.sync.dma_start(
                            out[c0:c0 + 4, msl, :].rearrange("c p w -> p c w"),
                            ob4)
    nc.compile()
    return nc


_PROGRAM = None


def kernel(_trace=False, **inputs):
    global _PROGRAM
    np_in = {k: np.ascontiguousarray(np.asarray(v)) for k, v in inputs.items()}
    g, x = np_in["g"], np_in["x"]
    consts = build_consts()
    in_maps = []
    for k in range(N_CORES):
        b, grp = k // 4, k % 4
        sl = slice(grp * NCH, (grp + 1) * NCH)
        m = dict(
            gb=np.ascontiguousarray(g[b].astype(np.float16)),
            xb=np.ascontiguousarray(x[b].astype(np.float16)),
            xres=np.ascontiguousarray(x[b][sl]),
            wgT=np.ascontiguousarray(np_in["wg_conv"][sl].T.astype(np.float16)),
            wxT=np.ascontiguousarray(np_in["wx_conv"][sl].T.astype(np.float16)),
            bg=np.ascontiguousarray(np.tile(np_in["bg_conv"][sl], 4)[:, None]),
            bx=np.ascontiguousarray(np.tile(np_in["bx_conv"][sl], 4)[:, None]),
            fpg=np.ascontiguousarray(np.moveaxis(
                np_in["filt_g"][sl], 3, 1).astype(np.float16)),
            fpx=np.ascontiguousarray(np.moveaxis(
                np_in["filt_x"][sl], 3, 1).astype(np.float16)),
            gamma=np.ascontiguousarray(np_in["ln_gamma"][sl][None, :]),
            beta=np.ascontiguousarray(np_in["ln_beta"][sl][None, :]),
            **consts,
        )
        in_maps.append(m)
    if _PROGRAM is None:
        _PROGRAM = build_program()
    res = run_bass_kernel_spmd(_PROGRAM, in_maps, core_ids=list(range(N_CORES)),
                               trace=_trace)
    out = np.zeros((B, C, H, W), np.float32)
    for k in range(N_CORES):
        b, grp = k // 4, k % 4
        out[b, grp * NCH:(grp + 1) * NCH] = res.results[k]["out"]
    kernel.last_debug = {k2: res.results[0][k2]
                         for k2 in ("dbg_zg", "dbg_att", "dbg_rstd")
                         if k2 in res.results[0]}
    if _trace:
        kernel.last_results = res
    return out


if __name__ == "__main__":
    ins = {
        "g": np.random.randn(B, C, H, W).astype(np.float32),
        "x": np.random.randn(B, C, H, W).astype(np.float32),
        "wg_conv": (np.random.randn(C, C) * 0.05).astype(np.float32),
        "bg_conv": np.zeros(C, np.float32),
        "wx_conv": (np.random.randn(C, C) * 0.05).astype(np.float32),
        "bx_conv": np.zeros(C, np.float32),
        "filt_g": (np.random.randn(C, H, WF, 2) * 0.02).astype(np.float32),
        "filt_x": (np.random.randn(C, H, WF, 2) * 0.02).astype(np.float32),
        "ln_gamma": np.ones(C, np.float32),
        "ln_beta": np.zeros(C, np.float32),
    }
    o = kernel(**ins)
    print("kernel ran, out shape", o.shape)



# revision 16
# speedup vs baseline: 1.4584x; 1.4584x over previous
"""Trainium2 Bass kernel for nn_AttentionFilter (B=2,C=128,H=256,W=510).

Sharding: 8 cores = 2 batches x 4 channel-groups of 32. Per core:
  Phase A: 1x1 conv as fp8 DoubleRow matmul (inputs pair-packed on host),
    bias+1/16 rescale fused into the PSUM->SBUF copy, spill y fp16 to DRAM
    padded to W=512.
  Phase B per channel: xbar DMA-transpose y -> [w, h] tiles; rfft_w and
    fft_h as fp16 DFT matmuls; complex filter multiply as fp16 4x-mode
    scalar_tensor_tensor ops; variance via bn_stats + integer-magic rsqrt
    (no ACT table swaps); freq attention matmul fp16; sigmoid on ACT with
    fused 1/sqrt(2pi var) row scale writing fp8; ifft_i and irfft_j as fp8
    DoubleRow matmuls (x16-scaled fp8 DFT constants, rescale fused into
    copies); residual add fused with 1/16 rescale; LN partial stats in
    fp16 (S2 scaled by 1/256 to avoid overflow).
  Phase C: AllReduce of fp16 stats within each batch group, rsqrt via
    integer magic + Newton, per-channel normalize, fp16 output.
"""
import sys

sys.path.insert(0, "/opt/trn_rl_repo")

import numpy as np
import ml_dtypes

import concourse.bass as bass
import concourse.mybir as mybir
import concourse.tile as tile
from concourse import bacc
from concourse.bass_utils import run_bass_kernel_spmd

B, C, H, W = 2, 128, 256, 510
WF = 256
W2 = 512
NCH = 32  # channels per core
N_CORES = 8
F32 = mybir.dt.float32
F16 = mybir.dt.float16
F8 = mybir.dt.float8e4
I32 = mybir.dt.int32
AF = mybir.ActivationFunctionType
OP = mybir.AluOpType
DR = mybir.MatmulPerfMode.DoubleRow

E4M3 = ml_dtypes.float8_e4m3
CS = 16.0  # fp8 constant scale
RSQRT_MAGIC = 0x5F3759DF
INV_SQRT_2PI = float(1.0 / np.sqrt(2.0 * np.pi))


_CORR = None


def _corr_w():
    # irfft2 of the constant 0.5 field of atten (from sigmoid = 0.5+0.5tanh):
    # after ifft over i: 8*(1+1j) at m=0; irfft over j gives this w-profile.
    global _CORR
    if _CORR is None:
        AR = np.fft.irfft(np.eye(WF), n=W, axis=0, norm="ortho")
        AI = np.fft.irfft(1j * np.eye(WF), n=W, axis=0, norm="ortho")
        _CORR = 8.0 * (AR.sum(axis=1) + AI.sum(axis=1))
    return _CORR


def build_consts():
    Fw = np.fft.rfft(np.eye(W), axis=0, norm="ortho").T  # [W, WF] complex
    fw_pack = np.zeros((W2, W2), np.float16)
    fw_pack[:W, :WF] = Fw.real
    fw_pack[:W, WF:] = Fw.imag
    DH = np.fft.fft(np.eye(H), axis=0, norm="ortho")  # [kh, h]
    dht_r = np.ascontiguousarray(DH.real.T * CS).astype(E4M3)  # [h, kh]
    dht_i = np.ascontiguousarray(DH.imag.T * CS).astype(E4M3)
    dht_ni = np.ascontiguousarray((-DH.imag).T * CS).astype(E4M3)
    IDH = np.fft.ifft(np.eye(H), axis=0, norm="ortho")  # [m, i]
    # ifft DoubleRow moving consts [i, (mR|mI)], x16 for fp8
    ida = np.zeros((H, W2), np.float32)
    ida[:, :WF] = IDH.real.T * (CS * 0.5)
    ida[:, WF:] = IDH.imag.T * (CS * 0.5)
    idb = np.zeros((H, W2), np.float32)
    idb[:, :WF] = -IDH.imag.T * (CS * 0.5)
    idb[:, WF:] = IDH.real.T * (CS * 0.5)
    AR = np.fft.irfft(np.eye(WF), n=W, axis=0, norm="ortho")  # [w, j]
    AI = np.fft.irfft(1j * np.eye(WF), n=W, axis=0, norm="ortho")
    awr = np.zeros((WF, W2), np.float32)
    awr[:, :W] = AR.T * CS
    awi = np.zeros((WF, W2), np.float32)
    awi[:, :W] = AI.T * CS
    return dict(
        fw=fw_pack,
        dht_r=dht_r, dht_i=dht_i, dht_ni=dht_ni,
        idht_a=ida.astype(E4M3), idht_b=idb.astype(E4M3),
        awr8=awr.astype(E4M3), awi8=awi.astype(E4M3),
    )


def build_program():
    nc = bacc.Bacc("TRN2", target_bir_lowering=False, debug=False,
                   num_devices=N_CORES)

    def inp(name, shape, dt=F32):
        return nc.dram_tensor(name, list(shape), dt, kind="ExternalInput").ap()

    g8 = inp("g8", (C, H, W2), F8)
    x8 = inp("x8", (C, H, W2), F8)
    wgT8 = inp("wgT8", (C, NCH), F8)
    wxT8 = inp("wxT8", (C, NCH), F8)
    bg = inp("bg", (128, 1))
    bx = inp("bx", (128, 1))
    fw = inp("fw", (W2, W2), F16)
    dht_r = inp("dht_r", (H, H), F8)
    dht_i = inp("dht_i", (H, H), F8)
    dht_ni = inp("dht_ni", (H, H), F8)
    idht_a = inp("idht_a", (H, W2), F8)
    idht_b = inp("idht_b", (H, W2), F8)
    awr8 = inp("awr8", (WF, W2), F8)
    awi8 = inp("awi8", (WF, W2), F8)
    # filters packed per channel-pair: [16, 4(c,ri), 256kh, 256kw]
    fpg = inp("fpg", (NCH // 2, 4, H, WF), F16)
    fpx = inp("fpx", (NCH // 2, 4, H, WF), F16)
    xres = inp("xres", (NCH, H, W2), F16)
    gamma = inp("gamma", (1, NCH))
    beta = inp("beta", (1, NCH))
    out = nc.dram_tensor("out", [NCH, H, W2], F16, kind="ExternalOutput").ap()

    with tile.TileContext(nc) as tc:
        with (
            tc.tile_pool(name="consts", bufs=1) as consts,
            tc.tile_pool(name="dram", bufs=1, space="DRAM") as dram,
        ):
            # ---- constants into SBUF
            c_fw = consts.tile([128, 4, W2], F16)
            nc.sync.dma_start(c_fw, fw.rearrange("(wc p) n -> p wc n", p=128))

            def ld2(src, dt=F16, ncol=H):
                t = consts.tile([128, 2, ncol], dt, tag=f"c_{src.name}")
                nc.sync.dma_start(t, src.rearrange("(hc p) m -> p hc m", p=128))
                return t

            c_dhtr = ld2(dht_r, F8)
            c_dhti = ld2(dht_i, F8)
            c_dhtni = ld2(dht_ni, F8)
            c_ida = ld2(idht_a, F8, W2)
            c_idb = ld2(idht_b, F8, W2)
            c_awr = ld2(awr8, F8, W2)
            c_awi = ld2(awi8, F8, W2)
            c_wgT = consts.tile([C, NCH], F8, tag="c_wgT")
            nc.sync.dma_start(c_wgT, wgT8)
            c_wxT = consts.tile([C, NCH], F8, tag="c_wxT")
            nc.sync.dma_start(c_wxT, wxT8)
            c_bg = consts.tile([128, 1], F32, tag="c_bg")
            nc.sync.dma_start(c_bg, bg)
            c_bx = consts.tile([128, 1], F32, tag="c_bx")
            nc.sync.dma_start(c_bx, bx)
            c_gamma = consts.tile([128, NCH], F32, tag="c_gamma")
            nc.sync.dma_start(c_gamma, gamma.to_broadcast([128, NCH]))
            c_beta = consts.tile([128, NCH], F32, tag="c_beta")
            nc.sync.dma_start(c_beta, beta.to_broadcast([128, NCH]))

            # ---- DRAM scratch
            ysp_g = dram.tile([NCH, H, W2], F16, tag="ysp_g")
            ysp_x = dram.tile([NCH, H, W2], F16, tag="ysp_x")
            cc_in = dram.tile([128, 2048], F32, tag="cc_in")
            cc_out = dram.tile([128, 2048], F32, tag="cc_out")

            # ---- Phase A: 1x1 conv (fp8 DoubleRow), spill y fp16
            HB = 32
            with (
                tc.tile_pool(name="pa_in", bufs=2) as pa_in,
                tc.tile_pool(name="pa_out", bufs=3) as pa_out,
                tc.tile_pool(name="pa_ps", bufs=4, space="PSUM") as pa_ps,
            ):
                for srct, wTt, biast, yspt in (
                    (g8, c_wgT, c_bg, ysp_g),
                    (x8, c_wxT, c_bx, ysp_x),
                ):
                    for blk in range(H // HB):
                        h0 = blk * HB
                        rh = pa_in.tile([C, HB, W2], F8, tag="rh")
                        nc.sync.dma_start(rh, srct[:, h0:h0 + HB, :])
                        stag = pa_out.tile([128, HB // 4, W2], F16, tag="stag")
                        for i2 in range(HB // 4):
                            ps = pa_ps.tile([128, W2], F32, tag="cps")
                            for j in range(4):
                                nc.tensor.matmul(
                                    ps[32 * j:32 * (j + 1), :], wTt,
                                    rh[:, i2 * 4 + j, :],
                                    start=True, stop=True,
                                    tile_position=(0, 32 * j))
                            # y = ps/16 + bias, alternate ACT/DVE
                            if i2 % 2 == 0:
                                nc.scalar.activation(
                                    stag[:, i2, :], ps, AF.Identity,
                                    bias=biast, scale=1.0 / CS)
                            else:
                                nc.vector.tensor_scalar(
                                    out=stag[:, i2, :], in0=ps,
                                    scalar1=1.0 / CS, scalar2=biast,
                                    op0=OP.mult, op1=OP.add)
                        for j in range(4):
                            nc.sync.dma_start(
                                yspt[:, h0 + j:h0 + HB:4, :],
                                stag[32 * j:32 * (j + 1), :, :])

            # ---- Phase B: per-channel frequency pipeline
            with (
                tc.tile_pool(name="pb_yt", bufs=2) as pb_yt,
                tc.tile_pool(name="pb_yw", bufs=3) as pb_yw,
                tc.tile_pool(name="pb_zp", bufs=3) as pb_zp,
                tc.tile_pool(name="pb_z", bufs=2) as pb_z,
                tc.tile_pool(name="pb_f", bufs=2) as pb_f,
                tc.tile_pool(name="pb_sm", bufs=2) as pb_sm,
                tc.tile_pool(name="pb_att", bufs=2) as pb_att,
                tc.tile_pool(name="pb_inv", bufs=2) as pb_inv,
                tc.tile_pool(name="pb_x", bufs=2) as pb_x,
                tc.tile_pool(name="pb_acc", bufs=1) as pb_acc,
                tc.tile_pool(name="pb_pw", bufs=2, space="PSUM") as pb_pw,
                tc.tile_pool(name="pb_pf", bufs=3, space="PSUM") as pb_pf,
                tc.tile_pool(name="pb_pt", bufs=3, space="PSUM") as pb_pt,
            ):
                S1 = pb_acc.tile([128, 2, W2], F32, tag="S1")
                S2 = pb_acc.tile([128, 2, W2], F32, tag="S2")
                nc.vector.memset(S1, 0.0)
                nc.vector.memset(S2, 0.0)
                r_all = pb_acc.tile([128, NCH, 2, W2], F16, tag="r_all")

                ytT = {}
                fP = {}
                for c in range(NCH):
                    if c % 2 == 0:
                        # xbar transpose two channels at once + filters
                        for t, ysp, fpd in ((0, ysp_g, fpg), (1, ysp_x, fpx)):
                            yt = pb_yt.tile([128, 4, 2, H], F16, tag=f"yt{t}")
                            for wc in range(4):
                                nc.sync.dma_start_transpose(
                                    yt[:, wc, :, :].rearrange(
                                        "p c h -> p (c h)"),
                                    ysp[c:c + 2, :,
                                        wc * 128:(wc + 1) * 128].rearrange(
                                        "c h w -> (c h) w"))
                            ytT[t] = yt
                            fpt = pb_f.tile([128, 4, 2, WF], F16, tag=f"fp{t}")
                            nc.sync.dma_start(
                                fpt, fpd[c // 2].rearrange(
                                    "cr (khc p) k -> p cr khc k", p=128))
                            fP[t] = fpt
                        xc = pb_x.tile([128, 2, 2, W2], F16, tag="xc")
                        nc.sync.dma_start(
                            xc, xres[c:c + 2].rearrange(
                                "c (mc p) w -> p c mc w", p=128))
                    cc = c % 2
                    z = {}
                    for t in (0, 1):
                        yt = ytT[t]
                        # B1: rfft_w -> yw8R/I [128(h), hc, kw] fp8
                        yw8R = pb_yw.tile([128, 2, WF], F8, tag="yw8R")
                        yw8I = pb_yw.tile([128, 2, WF], F8, tag="yw8I")
                        for hc in range(2):
                            pw = pb_pw.tile([128, W2], F32, tag="pw")
                            hsl = slice(hc * 128, (hc + 1) * 128)
                            for wc in range(4):
                                nc.tensor.matmul(
                                    pw, yt[:, wc, cc, hsl], c_fw[:, wc, :],
                                    start=(wc == 0), stop=(wc == 3))
                            nc.scalar.copy(out=yw8R[:, hc, :], in_=pw[:, 0:WF])
                            nc.scalar.copy(out=yw8I[:, hc, :], in_=pw[:, WF:W2])
                        # B2: fft_h (complex, fp8 DoubleRow accumulation)
                        zRp = pb_zp.tile([128, 2, WF], F16, tag="zRp")
                        zIp = pb_zp.tile([128, 2, WF], F16, tag="zIp")
                        for khc in range(2):
                            pfR = pb_pf.tile([128, W2], F32, tag="pf")
                            pfI = pb_pf.tile([128, W2], F32, tag="pf")
                            ksl = slice(khc * 128, (khc + 1) * 128)
                            nc.tensor.matmul(
                                pfR[:, 0:WF], c_dhtr[:, :, ksl], yw8R,
                                start=True, stop=False, perf_mode=DR)
                            nc.tensor.matmul(
                                pfR[:, 0:WF], c_dhtni[:, :, ksl], yw8I,
                                start=False, stop=True, perf_mode=DR)
                            nc.tensor.matmul(
                                pfI[:, 0:WF], c_dhti[:, :, ksl], yw8R,
                                start=True, stop=False, perf_mode=DR)
                            nc.tensor.matmul(
                                pfI[:, 0:WF], c_dhtr[:, :, ksl], yw8I,
                                start=False, stop=True, perf_mode=DR)
                            nc.scalar.copy(out=zRp[:, khc, :], in_=pfR[:, 0:WF])
                            nc.scalar.copy(out=zIp[:, khc, :], in_=pfI[:, 0:WF])
                        # B3: filter multiply (fp16 4x STT ops)
                        fR = fP[t][:, 2 * cc, :, :]
                        fI = fP[t][:, 2 * cc + 1, :, :]

                        t1 = pb_sm.tile([128, 2, WF], F16, tag="t1")
                        t2 = pb_sm.tile([128, 2, WF], F16, tag="t2")
                        t3 = pb_sm.tile([128, 2, WF], F16, tag="t3")
                        t4 = pb_sm.tile([128, 2, WF], F16, tag="t4")
                        nc.vector.tensor_mul(t1, zRp, fR)
                        nc.vector.tensor_mul(t2, zIp, fI)
                        nc.vector.tensor_mul(t3, zRp, fI)
                        nc.vector.tensor_mul(t4, zIp, fR)
                        zR = pb_z.tile([128, 2, WF], F16, tag=f"zR{t}")
                        zI = pb_z.tile([128, 2, WF], F16, tag=f"zI{t}")
                        nc.vector.tensor_sub(zR, t1, t2)
                        nc.vector.tensor_add(zI, t3, t4)
                        z[(t, "R")] = zR
                        z[(t, "I")] = zI
                        if t == 1:
                            nzI = pb_z.tile([128, 2, WF], F16, tag="nzI")
                            nc.vector.tensor_scalar_mul(nzI, zI, -1.0)
                            z[(1, "nI")] = nzI

                    gR, gI = z[(0, "R")], z[(0, "I")]
                    xR, xI, nxI = z[(1, "R")], z[(1, "I")], z[(1, "nI")]
                    # B4: variance over kw per kh row -> rstd=1/sqrt(2pi var)
                    v2 = pb_sm.tile([128, 2], F32, tag="v2")
                    for khc in range(2):
                        st = pb_sm.tile([128, 2, 6], F32, tag="bst")
                        nc.vector.bn_stats(out=st[:, 0, :], in_=gR[:, khc, :])
                        nc.vector.bn_stats(out=st[:, 1, :], in_=gI[:, khc, :])
                        mvR = pb_sm.tile([128, 2], F32, tag="mvR")
                        mvI = pb_sm.tile([128, 2], F32, tag="mvI")
                        nc.vector.bn_aggr(out=mvR, in_=st[:, 0, :])
                        nc.vector.bn_aggr(out=mvI, in_=st[:, 1, :])
                        nc.vector.scalar_tensor_tensor(
                            out=v2[:, khc:khc + 1], in0=mvR[:, 1:2],
                            scalar=1.0, in1=mvI[:, 1:2],
                            op0=OP.bypass, op1=OP.add)
                    # integer-magic rsqrt + 1 Newton step (fold 1/sqrt(2pi))
                    ti = pb_sm.tile([128, 2], I32, tag="ti")
                    nc.vector.tensor_scalar(
                        out=ti, in0=v2.bitcast(I32), scalar1=1, scalar2=0,
                        op0=OP.arith_shift_right, op1=OP.bypass)
                    nc.vector.tensor_scalar(
                        out=ti, in0=ti, scalar1=-1, scalar2=RSQRT_MAGIC,
                        op0=OP.mult, op1=OP.add)
                    y0 = ti.bitcast(F32)
                    tn = pb_sm.tile([128, 2], F32, tag="tn")
                    nc.vector.scalar_tensor_tensor(
                        out=tn, in0=y0, scalar=1.0, in1=y0,
                        op0=OP.bypass, op1=OP.mult)
                    nc.vector.scalar_tensor_tensor(
                        out=tn, in0=tn, scalar=1.0, in1=v2,
                        op0=OP.bypass, op1=OP.mult)
                    nc.vector.tensor_scalar(
                        out=tn, in0=tn, scalar1=-0.5, scalar2=1.5,
                        op0=OP.mult, op1=OP.add)
                    rstd = pb_sm.tile([128, 2], F32, tag="rstd")
                    nc.vector.scalar_tensor_tensor(
                        out=rstd, in0=y0, scalar=INV_SQRT_2PI * 0.5, in1=tn,
                        op0=OP.mult, op1=OP.mult)
                    # B5: scores + sigmoid -> att8 (fp8)
                    a8R = pb_att.tile([128, 2, WF], F8, tag="a8R")
                    a8I = pb_att.tile([128, 2, WF], F8, tag="a8I")
                    for ic in range(2):
                        pscR = pb_pt.tile([128, W2], F32, tag="pt")
                        pscI = pb_pt.tile([128, W2], F32, tag="pt")
                        isl = slice(ic * 128, (ic + 1) * 128)
                        for khc in range(2):
                            nc.tensor.matmul(
                                pscR[:, 0:WF], gR[:, khc, isl], xR[:, khc, :],
                                start=(khc == 0), stop=False)
                            nc.tensor.matmul(
                                pscR[:, 0:WF], gI[:, khc, isl], nxI[:, khc, :],
                                start=False, stop=(khc == 1))
                            nc.tensor.matmul(
                                pscI[:, 0:WF], gR[:, khc, isl], xI[:, khc, :],
                                start=(khc == 0), stop=False)
                            nc.tensor.matmul(
                                pscI[:, 0:WF], gI[:, khc, isl], xR[:, khc, :],
                                start=False, stop=(khc == 1))
                        nc.scalar.activation(a8R[:, ic, :], pscR[:, 0:WF],
                                             AF.Tanh,
                                             scale=rstd[:, ic:ic + 1])
                        nc.scalar.activation(a8I[:, ic, :], pscI[:, 0:WF],
                                             AF.Tanh,
                                             scale=rstd[:, ic:ic + 1])
                    # B7: ifft over i (fp8 DoubleRow), inv = pv/16
                    inv8 = pb_inv.tile([128, 2, W2], F8, tag="inv8")
                    for jc in range(2):
                        pv = pb_pt.tile([128, W2], F32, tag="pt")
                        jsl = slice(jc * 128, (jc + 1) * 128)
                        nc.tensor.matmul(pv, a8R[:, :, jsl], c_ida,
                                         start=True, stop=False, perf_mode=DR)
                        nc.tensor.matmul(pv, a8I[:, :, jsl], c_idb,
                                         start=False, stop=True, perf_mode=DR)
                        nc.scalar.activation(inv8[:, jc, :], pv, AF.Identity,
                                             scale=1.0 / CS)
                    # B8: irfft over j (fp8 DoubleRow) + residual + stats
                    for mc in range(2):
                        pr = pb_pt.tile([128, W2], F32, tag="pt")
                        msl = slice(mc * 128, (mc + 1) * 128)
                        msl2 = slice(WF + mc * 128, WF + (mc + 1) * 128)
                        nc.tensor.matmul(pr, inv8[:, :, msl], c_awr,
                                         start=True, stop=False, perf_mode=DR)
                        nc.tensor.matmul(pr, inv8[:, :, msl2], c_awi,
                                         start=False, stop=True, perf_mode=DR)
                        rc = r_all[:, c, mc, :]
                        nc.vector.scalar_tensor_tensor(
                            out=rc, in0=pr, scalar=1.0 / CS,
                            in1=xc[:, cc, mc, :], op0=OP.mult, op1=OP.add)
                        nc.gpsimd.tensor_add(S1[:, mc, :], S1[:, mc, :], rc)
                        sq = pb_sm.tile([128, W2], F16, tag="sq")
                        nc.scalar.activation(sq, rc, AF.Square,
                                             scale=1.0 / 16.0)
                        nc.gpsimd.tensor_add(S2[:, mc, :], S2[:, mc, :], sq)

                # ---- Phase C: LN stats AllReduce + normalize
                nc.sync.dma_start(cc_in[:, 0:1024],
                                  S1.rearrange("p a b -> p (a b)"))
                nc.sync.dma_start(cc_in[:, 1024:2048],
                                  S2.rearrange("p a b -> p (a b)"))
                nc.gpsimd.collective_compute(
                    "AllReduce", OP.add,
                    replica_groups=[[0, 1, 2, 3], [4, 5, 6, 7]],
                    ins=[cc_in.opt()], outs=[cc_out.opt()])
                mu32 = pb_acc.tile([128, 1024], F32, tag="mu32")
                nc.sync.dma_start(mu32, cc_out[:, 0:1024])
                nc.vector.tensor_scalar_mul(mu32, mu32, 1.0 / C)
                e2 = pb_acc.tile([128, 1024], F32, tag="e2")
                nc.sync.dma_start(e2, cc_out[:, 1024:2048])
                nc.vector.tensor_scalar_mul(e2, e2, 256.0 / C)
                var = pb_acc.tile([128, 1024], F32, tag="var")
                nc.vector.scalar_tensor_tensor(
                    out=var, in0=mu32, scalar=-1.0, in1=mu32,
                    op0=OP.mult, op1=OP.mult)
                nc.vector.scalar_tensor_tensor(
                    out=var, in0=var, scalar=1e-6, in1=e2,
                    op0=OP.add, op1=OP.add)
                tiL = pb_acc.tile([128, 1024], I32, tag="tiL")
                nc.vector.tensor_scalar(
                    out=tiL, in0=var.bitcast(I32), scalar1=1, scalar2=0,
                    op0=OP.arith_shift_right, op1=OP.bypass)
                nc.vector.tensor_scalar(
                    out=tiL, in0=tiL, scalar1=-1, scalar2=RSQRT_MAGIC,
                    op0=OP.mult, op1=OP.add)
                y0L = tiL.bitcast(F32)
                tnL = pb_acc.tile([128, 1024], F32, tag="tnL")
                rsL = pb_acc.tile([128, 1024], F32, tag="rsL")
                for it in range(2):
                    nc.vector.scalar_tensor_tensor(
                        out=tnL, in0=y0L, scalar=1.0, in1=y0L,
                        op0=OP.bypass, op1=OP.mult)
                    nc.vector.scalar_tensor_tensor(
                        out=tnL, in0=tnL, scalar=1.0, in1=var,
                        op0=OP.bypass, op1=OP.mult)
                    nc.vector.tensor_scalar(
                        out=tnL, in0=tnL, scalar1=-0.5, scalar2=1.5,
                        op0=OP.mult, op1=OP.add)
                    nc.vector.scalar_tensor_tensor(
                        out=rsL, in0=y0L, scalar=1.0, in1=tnL,
                        op0=OP.bypass, op1=OP.mult)
                    y0L = rsL
                mu16 = pb_acc.tile([128, 2, W2], F16, tag="mu16")
                nc.vector.tensor_copy(
                    out=mu16.rearrange("p a b -> p (a b)"), in_=mu32)
                rs16 = pb_acc.tile([128, 2, W2], F16, tag="rs16")
                nc.vector.tensor_copy(
                    out=rs16.rearrange("p a b -> p (a b)"), in_=rsL)
                with tc.tile_pool(name="pc_o", bufs=3) as pc_o:
                    for c in range(NCH):
                        ob = pc_o.tile([128, 2, W2], F16, tag="ob")
                        for mc in range(2):
                            tt = pc_o.tile([128, W2], F16, tag="tt")
                            nc.vector.tensor_sub(tt, r_all[:, c, mc, :],
                                                 mu16[:, mc, :])
                            nc.vector.tensor_mul(tt, tt, rs16[:, mc, :])
                            nc.vector.tensor_scalar(
                                out=ob[:, mc, :], in0=tt,
                                scalar1=c_gamma[:, c:c + 1],
                                scalar2=c_beta[:, c:c + 1],
                                op0=OP.mult, op1=OP.add)
                        nc.sync.dma_start(
                            out[c].rearrange("(mc p) w -> p mc w", p=128), ob)
    nc.compile()
    return nc


_PROGRAM = None


def kernel(_trace=False, **inputs):
    global _PROGRAM
    np_in = {k: np.ascontiguousarray(np.asarray(v)) for k, v in inputs.items()}
    g, x = np_in["g"], np_in["x"]
    consts = build_consts()

    def pack_gx(a):
        # [128, H, W] -> [128, H, 512] fp8 (pad w)
        p = np.zeros((C, H, W2), E4M3)
        p[:, :, :W] = a.astype(E4M3)
        return p

    def pack_w(wc, sl):
        # lhsT [128 c_in, 32] fp8 x16
        return np.ascontiguousarray((wc[sl].T * CS).astype(E4M3))

    def pack_filt(f):
        # f [32, H, WF, 2] -> [16, 4(c,ri), H, WF] fp16, pre-divided by CS
        # to undo the x16 fp8 scaling of the dht constants.
        m = np.moveaxis(f, 3, 1) * (1.0 / CS)  # [32, 2, H, WF]
        return np.ascontiguousarray(
            m.reshape(16, 4, H, WF).astype(np.float16))

    in_maps = []
    for k in range(N_CORES):
        b, grp = k // 4, k % 4
        sl = slice(grp * NCH, (grp + 1) * NCH)
        xr = np.zeros((NCH, H, W2), np.float16)
        xr[:, :, :W] = x[b][sl].astype(np.float16)
        xr[:, 0, :W] += _corr_w().astype(np.float16)
        m = dict(
            g8=pack_gx(g[b]),
            x8=pack_gx(x[b]),
            wgT8=pack_w(np_in["wg_conv"], sl),
            wxT8=pack_w(np_in["wx_conv"], sl),
            bg=np.ascontiguousarray(
                np.tile(np_in["bg_conv"][sl], 4)[:, None]).astype(np.float32),
            bx=np.ascontiguousarray(
                np.tile(np_in["bx_conv"][sl], 4)[:, None]).astype(np.float32),
            fpg=pack_filt(np_in["filt_g"][sl]),
            fpx=pack_filt(np_in["filt_x"][sl]),
            xres=xr,
            gamma=np.ascontiguousarray(
                np_in["ln_gamma"][sl][None, :]).astype(np.float32),
            beta=np.ascontiguousarray(
                np_in["ln_beta"][sl][None, :]).astype(np.float32),
            **consts,
        )
        in_maps.append(m)
    if _PROGRAM is None:
        _PROGRAM = build_program()
    res = run_bass_kernel_spmd(_PROGRAM, in_maps, core_ids=list(range(N_CORES)),
                               trace=_trace)
    outf = np.zeros((B, C, H, W), np.float32)
    for k in range(N_CORES):
        b, grp = k // 4, k % 4
        outf[b, grp * NCH:(grp + 1) * NCH] = \
            res.results[k]["out"][:, :, :W].astype(np.float32)
    if _trace:
        kernel.last_results = res
    return outf


if __name__ == "__main__":
    ins = {
        "g": np.random.randn(B, C, H, W).astype(np.float32),
        "x": np.random.randn(B, C, H, W).astype(np.float32),
        "wg_conv": (np.random.randn(C, C) * 0.05).astype(np.float32),
        "bg_conv": np.zeros(C, np.float32),
        "wx_conv": (np.random.randn(C, C) * 0.05).astype(np.float32),
        "bx_conv": np.zeros(C, np.float32),
        "filt_g": (np.random.randn(C, H, WF, 2) * 0.02).astype(np.float32),
        "filt_x": (np.random.randn(C, H, WF, 2) * 0.02).astype(np.float32),
        "ln_gamma": np.ones(C, np.float32),
        "ln_beta": np.zeros(C, np.float32),
    }
    o = kernel(**ins)
    print("kernel ran, out shape", o.shape)


# revision 19
# speedup vs baseline: 1.4887x; 1.0208x over previous
"""Trainium2 Bass kernel for nn_AttentionFilter (B=2,C=128,H=256,W=510).

Sharding: 8 cores = 2 batches x 4 channel-groups of 32. Per core:
  Phase A: 1x1 conv as fp8 matmul (x16-scaled fp8 weights, 1/16+bias fused
    into the PSUM->SBUF copy), spill y as fp8 to DRAM padded to W=512.
  Phase B (software-pipelined, backend of channel c-1 interleaved with
    frontend of channel c): xbar DMA-transpose of fp8 y viewed as uint16
    pairs -> pair-interleaved [w/2, 2, h] stationaries; rfft_w and fft_h as
    fp8 DoubleRow matmuls (x16-scaled fp8 DFT constants, rescale fused into
    PSUM copies); complex filter multiply as fp16 2x-mode tensor_tensor;
    variance via bn_stats + integer-magic rsqrt on Pool; freq attention
    matmul fp16 into a single PSUM bank; tanh on ACT (sigmoid = 0.5 +
    0.5tanh, the 0.5-DC correction folded into xres on host) with fused
    rstd/2 row scale writing fp8; ifft_i and irfft_j as fp8 DoubleRow
    matmuls; residual add fused with 1/16 rescale; LN stats: S1/S2 fp32
    accumulated on Pool, squares via ACT (x1/256 to bound range).
  Phase C: fp32 AllReduce of LN stats within each batch group, rsqrt via
    integer magic + 2 Newton steps, per-channel normalize, fp16 output.
"""
import sys

sys.path.insert(0, "/opt/trn_rl_repo")

import numpy as np
import ml_dtypes

import concourse.bass as bass
import concourse.mybir as mybir
import concourse.tile as tile
from concourse import bacc
from concourse.bass_utils import run_bass_kernel_spmd

B, C, H, W = 2, 128, 256, 510
WF = 256
W2 = 512
NCH = 32  # channels per core
N_CORES = 8
F32 = mybir.dt.float32
F16 = mybir.dt.float16
F8 = mybir.dt.float8e4
U16 = mybir.dt.uint16
I32 = mybir.dt.int32
AF = mybir.ActivationFunctionType
OP = mybir.AluOpType
DR = mybir.MatmulPerfMode.DoubleRow

E4M3 = ml_dtypes.float8_e4m3
CS = 16.0  # fp8 constant scale
RSQRT_MAGIC = 0x5F3759DF
INV_SQRT_2PI = float(1.0 / np.sqrt(2.0 * np.pi))

_CORR = None


def _corr_w():
    # irfft2 of the constant 0.5 field of atten (sigmoid = 0.5 + 0.5tanh):
    # after ifft over i it is 8*(1+1j) at m=0; irfft over j gives this
    # w-profile on the h=0 row.
    global _CORR
    if _CORR is None:
        AR = np.fft.irfft(np.eye(WF), n=W, axis=0, norm="ortho")
        AI = np.fft.irfft(1j * np.eye(WF), n=W, axis=0, norm="ortho")
        _CORR = 8.0 * (AR.sum(axis=1) + AI.sum(axis=1))
    return _CORR


def build_consts():
    Fw = np.fft.rfft(np.eye(W), axis=0, norm="ortho").T  # [W, WF] complex
    fw_pack = np.zeros((W2, W2), np.float32)
    fw_pack[:W, :WF] = Fw.real
    fw_pack[:W, WF:] = Fw.imag
    # pair-interleaved for DoubleRow: fw8[k, j, n] = fw_pack[2k+j, n] * CS
    fw8 = (fw_pack * CS).reshape(WF, 2, W2)
    DH = np.fft.fft(np.eye(H), axis=0, norm="ortho")  # [kh, h]
    dht_r = np.ascontiguousarray(DH.real.T * CS).astype(E4M3)  # [h, kh]
    dht_i = np.ascontiguousarray(DH.imag.T * CS).astype(E4M3)
    dht_ni = np.ascontiguousarray((-DH.imag).T * CS).astype(E4M3)
    IDH = np.fft.ifft(np.eye(H), axis=0, norm="ortho")  # [m, i]
    ida = np.zeros((H, W2), np.float32)
    ida[:, :WF] = IDH.real.T * (CS * 0.5)
    ida[:, WF:] = IDH.imag.T * (CS * 0.5)
    idb = np.zeros((H, W2), np.float32)
    idb[:, :WF] = -IDH.imag.T * (CS * 0.5)
    idb[:, WF:] = IDH.real.T * (CS * 0.5)
    AR = np.fft.irfft(np.eye(WF), n=W, axis=0, norm="ortho")  # [w, j]
    AI = np.fft.irfft(1j * np.eye(WF), n=W, axis=0, norm="ortho")
    awr = np.zeros((WF, W2), np.float32)
    awr[:, :W] = AR.T * CS
    awi = np.zeros((WF, W2), np.float32)
    awi[:, :W] = AI.T * CS
    return dict(
        fw8=fw8.astype(E4M3),
        dht_r=dht_r, dht_i=dht_i, dht_ni=dht_ni,
        idht_a=ida.astype(E4M3), idht_b=idb.astype(E4M3),
        awr8=awr.astype(E4M3), awi8=awi.astype(E4M3),
    )


def build_program():
    nc = bacc.Bacc("TRN2", target_bir_lowering=False, debug=False,
                   num_devices=N_CORES)

    def inp(name, shape, dt=F32):
        return nc.dram_tensor(name, list(shape), dt, kind="ExternalInput").ap()

    g8 = inp("g8", (C, H, W2), F8)
    x8 = inp("x8", (C, H, W2), F8)
    wgT8 = inp("wgT8", (C, NCH), F8)
    wxT8 = inp("wxT8", (C, NCH), F8)
    bg = inp("bg", (128, 1))
    bx = inp("bx", (128, 1))
    fw8 = inp("fw8", (WF, 2, W2), F8)
    dht_r = inp("dht_r", (H, H), F8)
    dht_i = inp("dht_i", (H, H), F8)
    dht_ni = inp("dht_ni", (H, H), F8)
    idht_a = inp("idht_a", (H, W2), F8)
    idht_b = inp("idht_b", (H, W2), F8)
    awr8 = inp("awr8", (WF, W2), F8)
    awi8 = inp("awi8", (WF, W2), F8)
    # filters packed per channel-pair: [16, 4(c,ri), 256kh, 256kw], x(1/CS)
    fpg = inp("fpg", (NCH // 2, 4, H, WF), F16)
    fpx = inp("fpx", (NCH // 2, 4, H, WF), F16)
    xres = inp("xres", (NCH, H, W2), F16)
    gamma = inp("gamma", (1, NCH))
    beta = inp("beta", (1, NCH))
    out = nc.dram_tensor("out", [NCH, H, W2], F16, kind="ExternalOutput").ap()

    with tile.TileContext(nc) as tc:
        with (
            tc.tile_pool(name="consts", bufs=1) as consts,
            tc.tile_pool(name="dram", bufs=1, space="DRAM") as dram,
        ):
            # ---- constants into SBUF
            c_fw = consts.tile([128, 2, 2, W2], F8, tag="c_fw")
            nc.sync.dma_start(
                c_fw, fw8.rearrange("(kc p) j n -> p kc j n", p=128))

            def ld2(src, ncol=H):
                t = consts.tile([128, 2, ncol], F8, tag=f"c_{src.name}")
                nc.sync.dma_start(t, src.rearrange("(hc p) m -> p hc m", p=128))
                return t

            c_dhtr = ld2(dht_r)
            c_dhti = ld2(dht_i)
            c_dhtni = ld2(dht_ni)
            c_ida = ld2(idht_a, W2)
            c_idb = ld2(idht_b, W2)
            c_awr = ld2(awr8, W2)
            c_awi = ld2(awi8, W2)
            c_wgT = consts.tile([C, NCH], F8, tag="c_wgT")
            nc.sync.dma_start(c_wgT, wgT8)
            c_wxT = consts.tile([C, NCH], F8, tag="c_wxT")
            nc.sync.dma_start(c_wxT, wxT8)
            c_bg = consts.tile([128, 1], F32, tag="c_bg")
            nc.sync.dma_start(c_bg, bg)
            c_bx = consts.tile([128, 1], F32, tag="c_bx")
            nc.sync.dma_start(c_bx, bx)
            c_gamma = consts.tile([128, NCH], F32, tag="c_gamma")
            nc.sync.dma_start(c_gamma, gamma.to_broadcast([128, NCH]))
            c_beta = consts.tile([128, NCH], F32, tag="c_beta")
            nc.sync.dma_start(c_beta, beta.to_broadcast([128, NCH]))

            # ---- DRAM scratch (y spill stored as uint16 fp8-pairs)
            ysp_g = dram.tile([NCH, H, WF], U16, tag="ysp_g")
            ysp_x = dram.tile([NCH, H, WF], U16, tag="ysp_x")
            cc_in = dram.tile([128, 2048], F32, tag="cc_in")
            cc_out = dram.tile([128, 2048], F32, tag="cc_out")

            # ---- Phase A: 1x1 conv (fp8), spill y fp8
            HB = 32
            with (
                tc.tile_pool(name="pa_in", bufs=2) as pa_in,
                tc.tile_pool(name="pa_out", bufs=3) as pa_out,
                tc.tile_pool(name="pa_ps", bufs=4, space="PSUM") as pa_ps,
            ):
                for srct, wTt, biast, yspt in (
                    (g8, c_wgT, c_bg, ysp_g),
                    (x8, c_wxT, c_bx, ysp_x),
                ):
                    for blk in range(H // HB):
                        h0 = blk * HB
                        rh = pa_in.tile([C, HB, W2], F8, tag="rh")
                        nc.sync.dma_start(rh, srct[:, h0:h0 + HB, :])
                        stag = pa_out.tile([128, HB // 4, W2], F8, tag="stag")
                        for i2 in range(HB // 4):
                            ps = pa_ps.tile([128, W2], F32, tag="cps")
                            for j in range(4):
                                nc.tensor.matmul(
                                    ps[32 * j:32 * (j + 1), :], wTt,
                                    rh[:, i2 * 4 + j, :],
                                    start=True, stop=True,
                                    tile_position=(0, 32 * j))
                            if i2 % 2 == 0:
                                nc.scalar.activation(
                                    stag[:, i2, :], ps, AF.Identity,
                                    bias=biast, scale=1.0 / CS)
                            else:
                                nc.vector.tensor_scalar(
                                    out=stag[:, i2, :], in0=ps,
                                    scalar1=1.0 / CS, scalar2=biast,
                                    op0=OP.mult, op1=OP.add)
                        stag16 = stag.bitcast(U16)
                        for j in range(4):
                            nc.sync.dma_start(
                                yspt[:, h0 + j:h0 + HB:4, :],
                                stag16[32 * j:32 * (j + 1), :, :])

            # ---- Phase B: software-pipelined per-channel pipeline
            with (
                tc.tile_pool(name="pb_yt", bufs=2) as pb_yt,
                tc.tile_pool(name="pb_yw", bufs=3) as pb_yw,
                tc.tile_pool(name="pb_zp", bufs=3) as pb_zp,
                tc.tile_pool(name="pb_z", bufs=2) as pb_z,
                tc.tile_pool(name="pb_f", bufs=2) as pb_f,
                tc.tile_pool(name="pb_sm", bufs=2) as pb_sm,
                tc.tile_pool(name="pb_att", bufs=2) as pb_att,
                tc.tile_pool(name="pb_inv", bufs=2) as pb_inv,
                tc.tile_pool(name="pb_x", bufs=2) as pb_x,
                tc.tile_pool(name="pb_acc", bufs=1) as pb_acc,
                tc.tile_pool(name="pb_pw", bufs=2, space="PSUM") as pb_pw,
                tc.tile_pool(name="pb_pf", bufs=2, space="PSUM") as pb_pf,
                tc.tile_pool(name="pb_pt", bufs=4, space="PSUM") as pb_pt,
            ):
                S1 = pb_acc.tile([128, 2, W2], F32, tag="S1")
                S2 = pb_acc.tile([128, 2, W2], F32, tag="S2")
                nc.vector.memset(S1, 0.0)
                nc.vector.memset(S2, 0.0)
                r_all = pb_acc.tile([128, NCH, 2, W2], F16, tag="r_all")

                ytT = {}
                fP = {}
                xc_d = {}

                def frontend(c):
                    cc = c % 2
                    if cc == 0:
                        for t, ysp, fpd in ((0, ysp_g, fpg), (1, ysp_x, fpx)):
                            yt = pb_yt.tile([128, 2, 2, WF], U16, tag=f"yt{t}")
                            for kc in range(2):
                                nc.sync.dma_start_transpose(
                                    yt[:, kc, :, :].rearrange(
                                        "p c h -> p (c h)"),
                                    ysp[c:c + 2, :,
                                        kc * 128:(kc + 1) * 128].rearrange(
                                        "c h w -> (c h) w"))
                            ytT[t] = yt
                            fpt = pb_f.tile([128, 4, 2, WF], F16, tag=f"fp{t}")
                            nc.sync.dma_start(
                                fpt, fpd[c // 2].rearrange(
                                    "cr (khc p) k -> p cr khc k", p=128))
                            fP[t] = fpt
                        xc = pb_x.tile([128, 2, 2, W2], F16, tag="xc")
                        nc.sync.dma_start(
                            xc, xres[c:c + 2].rearrange(
                                "c (mc p) w -> p c mc w", p=128))
                        xc_d[c // 2] = xc
                    z = {}
                    for t in (0, 1):
                        yt8 = ytT[t].bitcast(F8)  # [128, 2kc, 2c, 512]
                        # B1: rfft_w as fp8 DoubleRow over w-pairs
                        yw8 = pb_yw.tile([128, 2, W2], F8, tag="yw8")
                        for hc in range(2):
                            pw = pb_pw.tile([128, W2], F32, tag="pw")
                            first = True
                            for kc in range(2):
                                lhsv = yt8[:, kc, cc, :].rearrange(
                                    "p (h j) -> p j h", j=2)
                                for j in range(2):
                                    nc.tensor.matmul(
                                        pw,
                                        lhsv[:, j,
                                             hc * 128:(hc + 1) * 128],
                                        c_fw[:, kc, j, :],
                                        start=first,
                                        stop=(kc == 1 and j == 1))
                                    first = False
                            nc.scalar.activation(
                                yw8[:, hc, :], pw, AF.Identity,
                                scale=1.0 / CS)
                        # B2: fft_h as fp8 DoubleRow, R and I into one bank
                        ywR = yw8[:, :, 0:WF]
                        ywI = yw8[:, :, WF:W2]
                        zP = pb_zp.tile([128, 2, W2], F16, tag="zP")
                        for khc in range(2):
                            pf = pb_pf.tile([128, W2], F32, tag="pf")
                            ksl = slice(khc * 128, (khc + 1) * 128)
                            nc.tensor.matmul(
                                pf[:, 0:WF], c_dhtr[:, :, ksl], ywR,
                                start=True, stop=False, perf_mode=DR,
                                skip_group_check=True)
                            nc.tensor.matmul(
                                pf[:, 0:WF], c_dhtni[:, :, ksl], ywI,
                                start=False, stop=False, perf_mode=DR,
                                skip_group_check=True)
                            nc.tensor.matmul(
                                pf[:, WF:W2], c_dhti[:, :, ksl], ywR,
                                start=False, stop=False, perf_mode=DR,
                                skip_group_check=True)
                            nc.tensor.matmul(
                                pf[:, WF:W2], c_dhtr[:, :, ksl], ywI,
                                start=False, stop=True, perf_mode=DR,
                                skip_group_check=True)
                            nc.scalar.copy(out=zP[:, khc, :], in_=pf)
                        # B3: filter multiply (fp16 2x TT ops)
                        zRp = zP[:, :, 0:WF]
                        zIp = zP[:, :, WF:W2]
                        fR = fP[t][:, 2 * cc, :, :]
                        fI = fP[t][:, 2 * cc + 1, :, :]
                        t1 = pb_sm.tile([128, 2, WF], F16, tag="t1")
                        t2 = pb_sm.tile([128, 2, WF], F16, tag="t2")
                        t3 = pb_sm.tile([128, 2, WF], F16, tag="t3")
                        t4 = pb_sm.tile([128, 2, WF], F16, tag="t4")
                        nc.vector.tensor_mul(t1, zRp, fR)
                        nc.vector.tensor_mul(t2, zIp, fI)
                        nc.vector.tensor_mul(t3, zRp, fI)
                        nc.vector.tensor_mul(t4, zIp, fR)
                        zR = pb_z.tile([128, 2, WF], F16, tag=f"zR{t}")
                        zI = pb_z.tile([128, 2, WF], F16, tag=f"zI{t}")
                        nc.vector.tensor_sub(zR, t1, t2)
                        nc.vector.tensor_add(zI, t3, t4)
                        z[(t, "R")] = zR
                        z[(t, "I")] = zI
                        if t == 1:
                            nzI = pb_z.tile([128, 2, WF], F16, tag="nzI")
                            nc.vector.tensor_scalar_mul(nzI, zI, -1.0)
                            z[(1, "nI")] = nzI
                    # B4: variance over kw per kh row -> rstd (Pool)
                    gR, gI = z[(0, "R")], z[(0, "I")]
                    v2 = pb_sm.tile([128, 2], F32, tag="v2")
                    for khc in range(2):
                        st = pb_sm.tile([128, 2, 6], F32, tag="bst")
                        nc.vector.bn_stats(out=st[:, 0, :], in_=gR[:, khc, :])
                        nc.vector.bn_stats(out=st[:, 1, :], in_=gI[:, khc, :])
                        mvR = pb_sm.tile([128, 2], F32, tag="mvR")
                        mvI = pb_sm.tile([128, 2], F32, tag="mvI")
                        nc.vector.bn_aggr(out=mvR, in_=st[:, 0, :])
                        nc.vector.bn_aggr(out=mvI, in_=st[:, 1, :])
                        nc.gpsimd.tensor_add(v2[:, khc:khc + 1],
                                             mvR[:, 1:2], mvI[:, 1:2])
                    ti = pb_sm.tile([128, 2], I32, tag="ti")
                    nc.vector.tensor_scalar(
                        out=ti, in0=v2.bitcast(I32), scalar1=1, scalar2=0,
                        op0=OP.arith_shift_right, op1=OP.bypass)
                    nc.vector.tensor_scalar(
                        out=ti, in0=ti, scalar1=-1, scalar2=RSQRT_MAGIC,
                        op0=OP.mult, op1=OP.add)
                    y0 = ti.bitcast(F32)
                    tn = pb_sm.tile([128, 2], F32, tag="tn")
                    nc.gpsimd.tensor_mul(tn, y0, y0)
                    nc.gpsimd.tensor_mul(tn, tn, v2)
                    nc.vector.tensor_scalar(
                        out=tn, in0=tn, scalar1=-0.5, scalar2=1.5,
                        op0=OP.mult, op1=OP.add)
                    rstd = pb_sm.tile([128, 2], F32, tag="rstd")
                    nc.gpsimd.tensor_mul(rstd, y0, tn)
                    nc.vector.tensor_scalar_mul(rstd, rstd,
                                                INV_SQRT_2PI * 0.5)
                    return z, rstd

                def backend1(c, z, rstd):
                    gR, gI = z[(0, "R")], z[(0, "I")]
                    xR, xI, nxI = z[(1, "R")], z[(1, "I")], z[(1, "nI")]
                    # B5: scores (fp16) into one PSUM bank + tanh -> fp8
                    a8 = pb_att.tile([128, 2, W2], F8, tag="a8")
                    for ic in range(2):
                        psc = pb_pt.tile([128, W2], F32, tag="pt")
                        isl = slice(ic * 128, (ic + 1) * 128)
                        for khc in range(2):
                            nc.tensor.matmul(
                                psc[:, 0:WF], gR[:, khc, isl], xR[:, khc, :],
                                start=(khc == 0), stop=False,
                                skip_group_check=True)
                            nc.tensor.matmul(
                                psc[:, 0:WF], gI[:, khc, isl], nxI[:, khc, :],
                                start=False, stop=False,
                                skip_group_check=True)
                            nc.tensor.matmul(
                                psc[:, WF:W2], gR[:, khc, isl], xI[:, khc, :],
                                start=False, stop=False,
                                skip_group_check=True)
                            nc.tensor.matmul(
                                psc[:, WF:W2], gI[:, khc, isl], xR[:, khc, :],
                                start=False, stop=(khc == 1),
                                skip_group_check=True)
                        nc.scalar.activation(a8[:, ic, :], psc, AF.Tanh,
                                             scale=rstd[:, ic:ic + 1])
                    return a8

                def backend2(c, a8):
                    cc = c % 2
                    xc = xc_d[c // 2]
                    # B7: ifft over i (fp8 DoubleRow), inv = pv/16
                    inv8 = pb_inv.tile([128, 2, W2], F8, tag="inv8")
                    for jc in range(2):
                        pv = pb_pt.tile([128, W2], F32, tag="pt")
                        jsl = slice(jc * 128, (jc + 1) * 128)
                        jsl2 = slice(WF + jc * 128, WF + (jc + 1) * 128)
                        nc.tensor.matmul(pv, a8[:, :, jsl], c_ida,
                                         start=True, stop=False, perf_mode=DR)
                        nc.tensor.matmul(pv, a8[:, :, jsl2], c_idb,
                                         start=False, stop=True, perf_mode=DR)
                        nc.scalar.activation(inv8[:, jc, :], pv, AF.Identity,
                                             scale=1.0 / CS)
                    # B8: irfft over j (fp8 DoubleRow) + residual + stats
                    for mc in range(2):
                        pr = pb_pt.tile([128, W2], F32, tag="pt")
                        msl = slice(mc * 128, (mc + 1) * 128)
                        msl2 = slice(WF + mc * 128, WF + (mc + 1) * 128)
                        nc.tensor.matmul(pr, inv8[:, :, msl], c_awr,
                                         start=True, stop=False, perf_mode=DR)
                        nc.tensor.matmul(pr, inv8[:, :, msl2], c_awi,
                                         start=False, stop=True, perf_mode=DR)
                        rc = r_all[:, c, mc, :]
                        nc.vector.scalar_tensor_tensor(
                            out=rc, in0=pr, scalar=1.0 / CS,
                            in1=xc[:, cc, mc, :], op0=OP.mult, op1=OP.add)
                        nc.gpsimd.tensor_add(S1[:, mc, :], S1[:, mc, :], rc)
                        sq = pb_sm.tile([128, W2], F16, tag="sq")
                        nc.scalar.activation(sq, rc, AF.Square,
                                             scale=1.0 / 16.0)
                        nc.gpsimd.tensor_add(S2[:, mc, :], S2[:, mc, :], sq)

                state = {}
                att_st = {}
                for c in range(NCH + 2):
                    if c < NCH:
                        state[c] = frontend(c)
                    if 1 <= c <= NCH:
                        att_st[c - 1] = backend1(c - 1, *state.pop(c - 1))
                    if c >= 2:
                        backend2(c - 2, att_st.pop(c - 2))

                # ---- Phase C: LN stats AllReduce + normalize
                nc.sync.dma_start(cc_in[:, 0:1024],
                                  S1.rearrange("p a b -> p (a b)"))
                nc.sync.dma_start(cc_in[:, 1024:2048],
                                  S2.rearrange("p a b -> p (a b)"))
                nc.gpsimd.collective_compute(
                    "AllReduce", OP.add,
                    replica_groups=[[0, 1, 2, 3], [4, 5, 6, 7]],
                    ins=[cc_in.opt()], outs=[cc_out.opt()])
                mu32 = pb_acc.tile([128, 1024], F32, tag="mu32")
                nc.sync.dma_start(mu32, cc_out[:, 0:1024])
                nc.vector.tensor_scalar_mul(mu32, mu32, 1.0 / C)
                e2 = pb_acc.tile([128, 1024], F32, tag="e2")
                nc.sync.dma_start(e2, cc_out[:, 1024:2048])
                nc.vector.tensor_scalar_mul(e2, e2, 256.0 / C)
                var = pb_acc.tile([128, 1024], F32, tag="var")
                nc.vector.scalar_tensor_tensor(
                    out=var, in0=mu32, scalar=-1.0, in1=mu32,
                    op0=OP.mult, op1=OP.mult)
                nc.vector.scalar_tensor_tensor(
                    out=var, in0=var, scalar=1e-6, in1=e2,
                    op0=OP.add, op1=OP.add)
                tiL = pb_acc.tile([128, 1024], I32, tag="tiL")
                nc.vector.tensor_scalar(
                    out=tiL, in0=var.bitcast(I32), scalar1=1, scalar2=0,
                    op0=OP.arith_shift_right, op1=OP.bypass)
                nc.vector.tensor_scalar(
                    out=tiL, in0=tiL, scalar1=-1, scalar2=RSQRT_MAGIC,
                    op0=OP.mult, op1=OP.add)
                y0L = tiL.bitcast(F32)
                tnL = pb_acc.tile([128, 1024], F32, tag="tnL")
                rsL = pb_acc.tile([128, 1024], F32, tag="rsL")
                for it in range(2):
                    nc.vector.scalar_tensor_tensor(
                        out=tnL, in0=y0L, scalar=1.0, in1=y0L,
                        op0=OP.bypass, op1=OP.mult)
                    nc.vector.scalar_tensor_tensor(
                        out=tnL, in0=tnL, scalar=1.0, in1=var,
                        op0=OP.bypass, op1=OP.mult)
                    nc.vector.tensor_scalar(
                        out=tnL, in0=tnL, scalar1=-0.5, scalar2=1.5,
                        op0=OP.mult, op1=OP.add)
                    nc.vector.scalar_tensor_tensor(
                        out=rsL, in0=y0L, scalar=1.0, in1=tnL,
                        op0=OP.bypass, op1=OP.mult)
                    y0L = rsL
                mu16 = pb_acc.tile([128, 2, W2], F16, tag="mu16")
                nc.vector.tensor_copy(
                    out=mu16.rearrange("p a b -> p (a b)"), in_=mu32)
                rs16 = pb_acc.tile([128, 2, W2], F16, tag="rs16")
                nc.vector.tensor_copy(
                    out=rs16.rearrange("p a b -> p (a b)"), in_=rsL)
                with tc.tile_pool(name="pc_o", bufs=3) as pc_o:
                    for c in range(NCH):
                        ob = pc_o.tile([128, 2, W2], F16, tag="ob")
                        for mc in range(2):
                            tt = pc_o.tile([128, W2], F16, tag="tt")
                            nc.gpsimd.tensor_sub(tt, r_all[:, c, mc, :],
                                                 mu16[:, mc, :])
                            nc.vector.tensor_mul(tt, tt, rs16[:, mc, :])
                            nc.vector.tensor_scalar(
                                out=ob[:, mc, :], in0=tt,
                                scalar1=c_gamma[:, c:c + 1],
                                scalar2=c_beta[:, c:c + 1],
                                op0=OP.mult, op1=OP.add)
                        nc.sync.dma_start(
                            out[c].rearrange("(mc p) w -> p mc w", p=128), ob)
    nc.compile()
    return nc


_PROGRAM = None


def kernel(_trace=False, **inputs):
    global _PROGRAM
    np_in = {k: np.ascontiguousarray(np.asarray(v)) for k, v in inputs.items()}
    g, x = np_in["g"], np_in["x"]
    consts = build_consts()

    def pack_gx(a):
        p = np.zeros((C, H, W2), E4M3)
        p[:, :, :W] = a.astype(E4M3)
        return p

    def pack_w(wc, sl):
        return np.ascontiguousarray((wc[sl].T * CS).astype(E4M3))

    def pack_filt(f):
        # f [32, H, WF, 2] -> [16, 4(c,ri), H, WF] fp16, pre-divided by CS
        # to undo the x16 fp8 scaling of the dht constants.
        m = np.moveaxis(f, 3, 1) * (1.0 / CS)  # [32, 2, H, WF]
        return np.ascontiguousarray(
            m.reshape(16, 4, H, WF).astype(np.float16))

    in_maps = []
    for k in range(N_CORES):
        b, grp = k // 4, k % 4
        sl = slice(grp * NCH, (grp + 1) * NCH)
        xr = np.zeros((NCH, H, W2), np.float16)
        xr[:, :, :W] = x[b][sl].astype(np.float16)
        xr[:, 0, :W] += _corr_w().astype(np.float16)
        m = dict(
            g8=pack_gx(g[b]),
            x8=pack_gx(x[b]),
            wgT8=pack_w(np_in["wg_conv"], sl),
            wxT8=pack_w(np_in["wx_conv"], sl),
            bg=np.ascontiguousarray(
                np.tile(np_in["bg_conv"][sl], 4)[:, None]).astype(np.float32),
            bx=np.ascontiguousarray(
                np.tile(np_in["bx_conv"][sl], 4)[:, None]).astype(np.float32),
            fpg=pack_filt(np_in["filt_g"][sl]),
            fpx=pack_filt(np_in["filt_x"][sl]),
            xres=xr,
            gamma=np.ascontiguousarray(
                np_in["ln_gamma"][sl][None, :]).astype(np.float32),
            beta=np.ascontiguousarray(
                np_in["ln_beta"][sl][None, :]).astype(np.float32),
            **consts,
        )
        in_maps.append(m)
    if _PROGRAM is None:
        _PROGRAM = build_program()
    res = run_bass_kernel_spmd(_PROGRAM, in_maps, core_ids=list(range(N_CORES)),
                               trace=_trace)
    outf = np.zeros((B, C, H, W), np.float32)
    for k in range(N_CORES):
        b, grp = k // 4, k % 4
        outf[b, grp * NCH:(grp + 1) * NCH] = \
            res.results[k]["out"][:, :, :W].astype(np.float32)
    if _trace:
        kernel.last_results = res
    return outf


if __name__ == "__main__":
    ins = {
        "g": np.random.randn(B, C, H, W).astype(np.float32),
        "x": np.random.randn(B, C, H, W).astype(np.float32),
        "wg_conv": (np.random.randn(C, C) * 0.05).astype(np.float32),
        "bg_conv": np.zeros(C, np.float32),
        "wx_conv": (np.random.randn(C, C) * 0.05).astype(np.float32),
        "bx_conv": np.zeros(C, np.float32),
        "filt_g": (np.random.randn(C, H, WF, 2) * 0.02).astype(np.float32),
        "filt_x": (np.random.randn(C, H, WF, 2) * 0.02).astype(np.float32),
        "ln_gamma": np.ones(C, np.float32),
        "ln_beta": np.zeros(C, np.float32),
    }
    o = kernel(**ins)
    print("kernel ran, out shape", o.shape)


# revision 20
# speedup vs baseline: 1.5010x; 1.0082x over previous
"""Trainium2 Bass kernel for nn_AttentionFilter (B=2,C=128,H=256,W=510).

Sharding: 8 cores = 2 batches x 4 channel-groups of 32. Per core:
  Phase A: 1x1 conv as fp8 matmul (x16-scaled fp8 weights, 1/16+bias fused
    into the PSUM->SBUF copy), spill y as fp8 to DRAM padded to W=512.
  Phase B (software-pipelined, backend of channel c-1 interleaved with
    frontend of channel c): xbar DMA-transpose of fp8 y viewed as uint16
    pairs -> pair-interleaved [w/2, 2, h] stationaries; rfft_w and fft_h as
    fp8 DoubleRow matmuls (x16-scaled fp8 DFT constants, rescale fused into
    PSUM copies); complex filter multiply as fp16 2x-mode tensor_tensor;
    variance via bn_stats + integer-magic rsqrt on Pool; freq attention
    matmul fp16 into a single PSUM bank; tanh on ACT (sigmoid = 0.5 +
    0.5tanh, the 0.5-DC correction folded into xres on host) with fused
    rstd/2 row scale writing fp8; ifft_i and irfft_j as fp8 DoubleRow
    matmuls; residual add fused with 1/16 rescale; LN stats: S1/S2 fp32
    accumulated on Pool, squares via ACT (x1/256 to bound range).
  Phase C: fp32 AllReduce of LN stats within each batch group, rsqrt via
    integer magic + 2 Newton steps, per-channel normalize, fp16 output.
"""
import sys

sys.path.insert(0, "/opt/trn_rl_repo")

import numpy as np
import ml_dtypes

import concourse.bass as bass
import concourse.mybir as mybir
import concourse.tile as tile
from concourse import bacc
from concourse.bass_utils import run_bass_kernel_spmd

B, C, H, W = 2, 128, 256, 510
WF = 256
W2 = 512
NCH = 32  # channels per core
N_CORES = 8
F32 = mybir.dt.float32
F16 = mybir.dt.float16
F8 = mybir.dt.float8e4
U16 = mybir.dt.uint16
I32 = mybir.dt.int32
AF = mybir.ActivationFunctionType
OP = mybir.AluOpType
DR = mybir.MatmulPerfMode.DoubleRow

E4M3 = ml_dtypes.float8_e4m3
CS = 16.0  # fp8 constant scale
RSQRT_MAGIC = 0x5F3759DF
INV_SQRT_2PI = float(1.0 / np.sqrt(2.0 * np.pi))

_CORR = None


def _corr_w():
    # irfft2 of the constant 0.5 field of atten (sigmoid = 0.5 + 0.5tanh):
    # after ifft over i it is 8*(1+1j) at m=0; irfft over j gives this
    # w-profile on the h=0 row.
    global _CORR
    if _CORR is None:
        AR = np.fft.irfft(np.eye(WF), n=W, axis=0, norm="ortho")
        AI = np.fft.irfft(1j * np.eye(WF), n=W, axis=0, norm="ortho")
        _CORR = 8.0 * (AR.sum(axis=1) + AI.sum(axis=1))
    return _CORR


def build_consts():
    Fw = np.fft.rfft(np.eye(W), axis=0, norm="ortho").T  # [W, WF] complex
    fw_pack = np.zeros((W2, W2), np.float32)
    fw_pack[:W, :WF] = Fw.real
    fw_pack[:W, WF:] = Fw.imag
    # pair-interleaved for DoubleRow: fw8[k, j, n] = fw_pack[2k+j, n] * CS
    fw8 = (fw_pack * CS).reshape(WF, 2, W2)
    DH = np.fft.fft(np.eye(H), axis=0, norm="ortho")  # [kh, h]
    dht_r = np.ascontiguousarray(DH.real.T * CS).astype(E4M3)  # [h, kh]
    dht_i = np.ascontiguousarray(DH.imag.T * CS).astype(E4M3)
    dht_ni = np.ascontiguousarray((-DH.imag).T * CS).astype(E4M3)
    IDH = np.fft.ifft(np.eye(H), axis=0, norm="ortho")  # [m, i]
    ida = np.zeros((H, W2), np.float32)
    ida[:, :WF] = IDH.real.T * (CS * 0.5)
    ida[:, WF:] = IDH.imag.T * (CS * 0.5)
    idb = np.zeros((H, W2), np.float32)
    idb[:, :WF] = -IDH.imag.T * (CS * 0.5)
    idb[:, WF:] = IDH.real.T * (CS * 0.5)
    AR = np.fft.irfft(np.eye(WF), n=W, axis=0, norm="ortho")  # [w, j]
    AI = np.fft.irfft(1j * np.eye(WF), n=W, axis=0, norm="ortho")
    awr = np.zeros((WF, W2), np.float32)
    awr[:, :W] = AR.T * CS
    awi = np.zeros((WF, W2), np.float32)
    awi[:, :W] = AI.T * CS
    return dict(
        fw8=fw8.astype(E4M3),
        dht_r=dht_r, dht_i=dht_i, dht_ni=dht_ni,
        idht_a=ida.astype(E4M3), idht_b=idb.astype(E4M3),
        awr8=awr.astype(E4M3), awi8=awi.astype(E4M3),
    )


def build_program():
    nc = bacc.Bacc("TRN2", target_bir_lowering=False, debug=False,
                   num_devices=N_CORES)

    def inp(name, shape, dt=F32):
        return nc.dram_tensor(name, list(shape), dt, kind="ExternalInput").ap()

    g8 = inp("g8", (C, H, W2), F8)
    x8 = inp("x8", (C, H, W2), F8)
    wgT8 = inp("wgT8", (C, NCH), F8)
    wxT8 = inp("wxT8", (C, NCH), F8)
    bg = inp("bg", (128, 1))
    bx = inp("bx", (128, 1))
    fw8 = inp("fw8", (WF, 2, W2), F8)
    dht_r = inp("dht_r", (H, H), F8)
    dht_i = inp("dht_i", (H, H), F8)
    dht_ni = inp("dht_ni", (H, H), F8)
    idht_a = inp("idht_a", (H, W2), F8)
    idht_b = inp("idht_b", (H, W2), F8)
    awr8 = inp("awr8", (WF, W2), F8)
    awi8 = inp("awi8", (WF, W2), F8)
    # filters packed per channel-pair: [16, 4(c,ri), 256kh, 256kw], x(1/CS)
    fpg = inp("fpg", (NCH // 2, 4, H, WF), F16)
    fpx = inp("fpx", (NCH // 2, 4, H, WF), F16)
    xres = inp("xres", (NCH, H, W2), F16)
    gamma = inp("gamma", (1, NCH))
    beta = inp("beta", (1, NCH))
    out = nc.dram_tensor("out", [NCH, H, W2], F16, kind="ExternalOutput").ap()

    with tile.TileContext(nc) as tc:
        with (
            tc.tile_pool(name="consts", bufs=1) as consts,
            tc.tile_pool(name="dram", bufs=1, space="DRAM") as dram,
        ):
            # ---- constants into SBUF
            c_fw = consts.tile([128, 2, 2, W2], F8, tag="c_fw")
            nc.sync.dma_start(
                c_fw, fw8.rearrange("(kc p) j n -> p kc j n", p=128))

            def ld2(src, ncol=H):
                t = consts.tile([128, 2, ncol], F8, tag=f"c_{src.name}")
                nc.sync.dma_start(t, src.rearrange("(hc p) m -> p hc m", p=128))
                return t

            c_dhtr = ld2(dht_r)
            c_dhti = ld2(dht_i)
            c_dhtni = ld2(dht_ni)
            c_ida = ld2(idht_a, W2)
            c_idb = ld2(idht_b, W2)
            c_awr = ld2(awr8, W2)
            c_awi = ld2(awi8, W2)
            c_wgT = consts.tile([C, NCH], F8, tag="c_wgT")
            nc.sync.dma_start(c_wgT, wgT8)
            c_wxT = consts.tile([C, NCH], F8, tag="c_wxT")
            nc.sync.dma_start(c_wxT, wxT8)
            c_bg = consts.tile([128, 1], F32, tag="c_bg")
            nc.sync.dma_start(c_bg, bg)
            c_bx = consts.tile([128, 1], F32, tag="c_bx")
            nc.sync.dma_start(c_bx, bx)
            c_gamma = consts.tile([128, NCH], F32, tag="c_gamma")
            nc.sync.dma_start(c_gamma, gamma.to_broadcast([128, NCH]))
            c_beta = consts.tile([128, NCH], F32, tag="c_beta")
            nc.sync.dma_start(c_beta, beta.to_broadcast([128, NCH]))

            # ---- DRAM scratch (y spill stored as uint16 fp8-pairs)
            ysp_g = dram.tile([NCH, H, WF], U16, tag="ysp_g")
            ysp_x = dram.tile([NCH, H, WF], U16, tag="ysp_x")
            cc_in = dram.tile([128, 2048], F32, tag="cc_in")
            cc_out = dram.tile([128, 2048], F32, tag="cc_out")

            # ---- Phase A: 1x1 conv (fp8), spill y fp8
            HB = 32
            with (
                tc.tile_pool(name="pa_in", bufs=3) as pa_in,
                tc.tile_pool(name="pa_out", bufs=3) as pa_out,
                tc.tile_pool(name="pa_ps", bufs=4, space="PSUM") as pa_ps,
            ):
                for srct, wTt, biast, yspt in (
                    (g8, c_wgT, c_bg, ysp_g),
                    (x8, c_wxT, c_bx, ysp_x),
                ):
                    for blk in range(H // HB):
                        h0 = blk * HB
                        rh = pa_in.tile([C, HB, W2], F8, tag="rh")
                        nc.sync.dma_start(rh, srct[:, h0:h0 + HB, :])
                        stag = pa_out.tile([128, HB // 4, W2], F8, tag="stag")
                        for i2 in range(HB // 4):
                            ps = pa_ps.tile([128, W2], F32, tag="cps")
                            for j in range(4):
                                nc.tensor.matmul(
                                    ps[32 * j:32 * (j + 1), :], wTt,
                                    rh[:, i2 * 4 + j, :],
                                    start=True, stop=True,
                                    tile_position=(0, 32 * j))
                            if i2 % 2 == 0:
                                nc.scalar.activation(
                                    stag[:, i2, :], ps, AF.Identity,
                                    bias=biast, scale=1.0 / CS)
                            else:
                                nc.vector.tensor_scalar(
                                    out=stag[:, i2, :], in0=ps,
                                    scalar1=1.0 / CS, scalar2=biast,
                                    op0=OP.mult, op1=OP.add)
                        stag16 = stag.bitcast(U16)
                        for j in range(4):
                            nc.sync.dma_start(
                                yspt[:, h0 + j:h0 + HB:4, :],
                                stag16[32 * j:32 * (j + 1), :, :])

            # ---- Phase B: software-pipelined per-channel pipeline
            with (
                tc.tile_pool(name="pb_yt", bufs=2) as pb_yt,
                tc.tile_pool(name="pb_yw", bufs=4) as pb_yw,
                tc.tile_pool(name="pb_zp", bufs=4) as pb_zp,
                tc.tile_pool(name="pb_z", bufs=3) as pb_z,
                tc.tile_pool(name="pb_f", bufs=2) as pb_f,
                tc.tile_pool(name="pb_sm", bufs=3) as pb_sm,
                tc.tile_pool(name="pb_att", bufs=3) as pb_att,
                tc.tile_pool(name="pb_inv", bufs=3) as pb_inv,
                tc.tile_pool(name="pb_x", bufs=2) as pb_x,
                tc.tile_pool(name="pb_acc", bufs=1) as pb_acc,
                tc.tile_pool(name="pb_pw", bufs=2, space="PSUM") as pb_pw,
                tc.tile_pool(name="pb_pf", bufs=2, space="PSUM") as pb_pf,
                tc.tile_pool(name="pb_pt", bufs=4, space="PSUM") as pb_pt,
            ):
                S1 = pb_acc.tile([128, 2, W2], F32, tag="S1")
                S2 = pb_acc.tile([128, 2, W2], F32, tag="S2")
                nc.vector.memset(S1, 0.0)
                nc.vector.memset(S2, 0.0)
                r_all = pb_acc.tile([128, NCH, 2, W2], F16, tag="r_all")

                ytT = {}
                fP = {}
                xc_d = {}

                def frontend(c):
                    cc = c % 2
                    if cc == 0:
                        for t, ysp, fpd in ((0, ysp_g, fpg), (1, ysp_x, fpx)):
                            yt = pb_yt.tile([128, 2, 2, WF], U16, tag=f"yt{t}")
                            for kc in range(2):
                                nc.sync.dma_start_transpose(
                                    yt[:, kc, :, :].rearrange(
                                        "p c h -> p (c h)"),
                                    ysp[c:c + 2, :,
                                        kc * 128:(kc + 1) * 128].rearrange(
                                        "c h w -> (c h) w"))
                            ytT[t] = yt
                            fpt = pb_f.tile([128, 4, 2, WF], F16, tag=f"fp{t}")
                            nc.sync.dma_start(
                                fpt, fpd[c // 2].rearrange(
                                    "cr (khc p) k -> p cr khc k", p=128))
                            fP[t] = fpt
                        xc = pb_x.tile([128, 2, 2, W2], F16, tag="xc")
                        nc.sync.dma_start(
                            xc, xres[c:c + 2].rearrange(
                                "c (mc p) w -> p c mc w", p=128))
                        xc_d[c // 2] = xc
                    z = {}
                    for t in (0, 1):
                        yt8 = ytT[t].bitcast(F8)  # [128, 2kc, 2c, 512]
                        # B1: rfft_w as fp8 DoubleRow over w-pairs
                        yw8 = pb_yw.tile([128, 2, W2], F8, tag="yw8")
                        for hc in range(2):
                            pw = pb_pw.tile([128, W2], F32, tag="pw")
                            first = True
                            for kc in range(2):
                                lhsv = yt8[:, kc, cc, :].rearrange(
                                    "p (h j) -> p j h", j=2)
                                for j in range(2):
                                    nc.tensor.matmul(
                                        pw,
                                        lhsv[:, j,
                                             hc * 128:(hc + 1) * 128],
                                        c_fw[:, kc, j, :],
                                        start=first,
                                        stop=(kc == 1 and j == 1))
                                    first = False
                            nc.scalar.activation(
                                yw8[:, hc, :], pw, AF.Identity,
                                scale=1.0 / CS)
                        # B2: fft_h as fp8 DoubleRow, R and I into one bank
                        ywR = yw8[:, :, 0:WF]
                        ywI = yw8[:, :, WF:W2]
                        zP = pb_zp.tile([128, 2, W2], F16, tag="zP")
                        for khc in range(2):
                            pf = pb_pf.tile([128, W2], F32, tag="pf")
                            ksl = slice(khc * 128, (khc + 1) * 128)
                            nc.tensor.matmul(
                                pf[:, 0:WF], c_dhtr[:, :, ksl], ywR,
                                start=True, stop=False, perf_mode=DR,
                                skip_group_check=True)
                            nc.tensor.matmul(
                                pf[:, 0:WF], c_dhtni[:, :, ksl], ywI,
                                start=False, stop=False, perf_mode=DR,
                                skip_group_check=True)
                            nc.tensor.matmul(
                                pf[:, WF:W2], c_dhti[:, :, ksl], ywR,
                                start=False, stop=False, perf_mode=DR,
                                skip_group_check=True)
                            nc.tensor.matmul(
                                pf[:, WF:W2], c_dhtr[:, :, ksl], ywI,
                                start=False, stop=True, perf_mode=DR,
                                skip_group_check=True)
                            nc.scalar.copy(out=zP[:, khc, :], in_=pf)
                        # B3: filter multiply (fp16 2x TT ops)
                        zRp = zP[:, :, 0:WF]
                        zIp = zP[:, :, WF:W2]
                        fR = fP[t][:, 2 * cc, :, :]
                        fI = fP[t][:, 2 * cc + 1, :, :]
                        t1 = pb_sm.tile([128, 2, WF], F16, tag="t1")
                        t2 = pb_sm.tile([128, 2, WF], F16, tag="t2")
                        t3 = pb_sm.tile([128, 2, WF], F16, tag="t3")
                        t4 = pb_sm.tile([128, 2, WF], F16, tag="t4")
                        nc.vector.tensor_mul(t1, zRp, fR)
                        nc.vector.tensor_mul(t2, zIp, fI)
                        nc.vector.tensor_mul(t3, zRp, fI)
                        nc.vector.tensor_mul(t4, zIp, fR)
                        zR = pb_z.tile([128, 2, WF], F16, tag=f"zR{t}")
                        zI = pb_z.tile([128, 2, WF], F16, tag=f"zI{t}")
                        nc.vector.tensor_sub(zR, t1, t2)
                        nc.vector.tensor_add(zI, t3, t4)
                        z[(t, "R")] = zR
                        z[(t, "I")] = zI
                        if t == 1:
                            nzI = pb_z.tile([128, 2, WF], F16, tag="nzI")
                            nc.vector.tensor_scalar_mul(nzI, zI, -1.0)
                            z[(1, "nI")] = nzI
                    # B4: variance over kw per kh row -> rstd (Pool)
                    gR, gI = z[(0, "R")], z[(0, "I")]
                    v2 = pb_sm.tile([128, 2], F32, tag="v2")
                    for khc in range(2):
                        st = pb_sm.tile([128, 2, 6], F32, tag="bst")
                        nc.vector.bn_stats(out=st[:, 0, :], in_=gR[:, khc, :])
                        nc.vector.bn_stats(out=st[:, 1, :], in_=gI[:, khc, :])
                        mvR = pb_sm.tile([128, 2], F32, tag="mvR")
                        mvI = pb_sm.tile([128, 2], F32, tag="mvI")
                        nc.vector.bn_aggr(out=mvR, in_=st[:, 0, :])
                        nc.vector.bn_aggr(out=mvI, in_=st[:, 1, :])
                        nc.gpsimd.tensor_add(v2[:, khc:khc + 1],
                                             mvR[:, 1:2], mvI[:, 1:2])
                    ti = pb_sm.tile([128, 2], I32, tag="ti")
                    nc.vector.tensor_scalar(
                        out=ti, in0=v2.bitcast(I32), scalar1=1, scalar2=0,
                        op0=OP.arith_shift_right, op1=OP.bypass)
                    nc.vector.tensor_scalar(
                        out=ti, in0=ti, scalar1=-1, scalar2=RSQRT_MAGIC,
                        op0=OP.mult, op1=OP.add)
                    y0 = ti.bitcast(F32)
                    tn = pb_sm.tile([128, 2], F32, tag="tn")
                    nc.gpsimd.tensor_mul(tn, y0, y0)
                    nc.gpsimd.tensor_mul(tn, tn, v2)
                    nc.vector.tensor_scalar(
                        out=tn, in0=tn, scalar1=-0.5, scalar2=1.5,
                        op0=OP.mult, op1=OP.add)
                    rstd = pb_sm.tile([128, 2], F32, tag="rstd")
                    nc.gpsimd.tensor_mul(rstd, y0, tn)
                    nc.vector.tensor_scalar_mul(rstd, rstd,
                                                INV_SQRT_2PI * 0.5)
                    return z, rstd

                def backend1(c, z, rstd):
                    gR, gI = z[(0, "R")], z[(0, "I")]
                    xR, xI, nxI = z[(1, "R")], z[(1, "I")], z[(1, "nI")]
                    # B5: scores (fp16) into one PSUM bank + tanh -> fp8
                    a8 = pb_att.tile([128, 2, W2], F8, tag="a8")
                    for ic in range(2):
                        psc = pb_pt.tile([128, W2], F32, tag="pt")
                        isl = slice(ic * 128, (ic + 1) * 128)
                        for khc in range(2):
                            nc.tensor.matmul(
                                psc[:, 0:WF], gR[:, khc, isl], xR[:, khc, :],
                                start=(khc == 0), stop=False,
                                skip_group_check=True)
                            nc.tensor.matmul(
                                psc[:, 0:WF], gI[:, khc, isl], nxI[:, khc, :],
                                start=False, stop=False,
                                skip_group_check=True)
                            nc.tensor.matmul(
                                psc[:, WF:W2], gR[:, khc, isl], xI[:, khc, :],
                                start=False, stop=False,
                                skip_group_check=True)
                            nc.tensor.matmul(
                                psc[:, WF:W2], gI[:, khc, isl], xR[:, khc, :],
                                start=False, stop=(khc == 1),
                                skip_group_check=True)
                        nc.scalar.activation(a8[:, ic, :], psc, AF.Tanh,
                                             scale=rstd[:, ic:ic + 1])
                    return a8

                def backend2(c, a8):
                    cc = c % 2
                    xc = xc_d[c // 2]
                    # B7: ifft over i (fp8 DoubleRow), inv = pv/16
                    inv8 = pb_inv.tile([128, 2, W2], F8, tag="inv8")
                    for jc in range(2):
                        pv = pb_pt.tile([128, W2], F32, tag="pt")
                        jsl = slice(jc * 128, (jc + 1) * 128)
                        jsl2 = slice(WF + jc * 128, WF + (jc + 1) * 128)
                        nc.tensor.matmul(pv, a8[:, :, jsl], c_ida,
                                         start=True, stop=False, perf_mode=DR)
                        nc.tensor.matmul(pv, a8[:, :, jsl2], c_idb,
                                         start=False, stop=True, perf_mode=DR)
                        nc.scalar.activation(inv8[:, jc, :], pv, AF.Identity,
                                             scale=1.0 / CS)
                    # B8: irfft over j (fp8 DoubleRow) + residual + stats
                    for mc in range(2):
                        pr = pb_pt.tile([128, W2], F32, tag="pt")
                        msl = slice(mc * 128, (mc + 1) * 128)
                        msl2 = slice(WF + mc * 128, WF + (mc + 1) * 128)
                        nc.tensor.matmul(pr, inv8[:, :, msl], c_awr,
                                         start=True, stop=False, perf_mode=DR)
                        nc.tensor.matmul(pr, inv8[:, :, msl2], c_awi,
                                         start=False, stop=True, perf_mode=DR)
                        rc = r_all[:, c, mc, :]
                        nc.vector.scalar_tensor_tensor(
                            out=rc, in0=pr, scalar=1.0 / CS,
                            in1=xc[:, cc, mc, :], op0=OP.mult, op1=OP.add)
                        nc.gpsimd.tensor_add(S1[:, mc, :], S1[:, mc, :], rc)
                        sq = pb_sm.tile([128, W2], F16, tag="sq")
                        nc.scalar.activation(sq, rc, AF.Square,
                                             scale=1.0 / 16.0)
                        nc.gpsimd.tensor_add(S2[:, mc, :], S2[:, mc, :], sq)

                state = {}
                att_st = {}
                for c in range(NCH + 2):
                    if c < NCH:
                        state[c] = frontend(c)
                    if 1 <= c <= NCH:
                        att_st[c - 1] = backend1(c - 1, *state.pop(c - 1))
                    if c >= 2:
                        backend2(c - 2, att_st.pop(c - 2))

                # ---- Phase C: LN stats AllReduce + normalize
                nc.sync.dma_start(cc_in[:, 0:1024],
                                  S1.rearrange("p a b -> p (a b)"))
                nc.sync.dma_start(cc_in[:, 1024:2048],
                                  S2.rearrange("p a b -> p (a b)"))
                nc.gpsimd.collective_compute(
                    "AllReduce", OP.add,
                    replica_groups=[[0, 1, 2, 3], [4, 5, 6, 7]],
                    ins=[cc_in.opt()], outs=[cc_out.opt()])
                mu32 = pb_acc.tile([128, 1024], F32, tag="mu32")
                nc.sync.dma_start(mu32, cc_out[:, 0:1024])
                nc.vector.tensor_scalar_mul(mu32, mu32, 1.0 / C)
                e2 = pb_acc.tile([128, 1024], F32, tag="e2")
                nc.sync.dma_start(e2, cc_out[:, 1024:2048])
                nc.vector.tensor_scalar_mul(e2, e2, 256.0 / C)
                var = pb_acc.tile([128, 1024], F32, tag="var")
                nc.vector.scalar_tensor_tensor(
                    out=var, in0=mu32, scalar=-1.0, in1=mu32,
                    op0=OP.mult, op1=OP.mult)
                nc.vector.scalar_tensor_tensor(
                    out=var, in0=var, scalar=1e-6, in1=e2,
                    op0=OP.add, op1=OP.add)
                tiL = pb_acc.tile([128, 1024], I32, tag="tiL")
                nc.vector.tensor_scalar(
                    out=tiL, in0=var.bitcast(I32), scalar1=1, scalar2=0,
                    op0=OP.arith_shift_right, op1=OP.bypass)
                nc.vector.tensor_scalar(
                    out=tiL, in0=tiL, scalar1=-1, scalar2=RSQRT_MAGIC,
                    op0=OP.mult, op1=OP.add)
                y0L = tiL.bitcast(F32)
                tnL = pb_acc.tile([128, 1024], F32, tag="tnL")
                rsL = pb_acc.tile([128, 1024], F32, tag="rsL")
                for it in range(2):
                    nc.vector.scalar_tensor_tensor(
                        out=tnL, in0=y0L, scalar=1.0, in1=y0L,
                        op0=OP.bypass, op1=OP.mult)
                    nc.vector.scalar_tensor_tensor(
                        out=tnL, in0=tnL, scalar=1.0, in1=var,
                        op0=OP.bypass, op1=OP.mult)
                    nc.vector.tensor_scalar(
                        out=tnL, in0=tnL, scalar1=-0.5, scalar2=1.5,
                        op0=OP.mult, op1=OP.add)
                    nc.vector.scalar_tensor_tensor(
                        out=rsL, in0=y0L, scalar=1.0, in1=tnL,
                        op0=OP.bypass, op1=OP.mult)
                    y0L = rsL
                mu16 = pb_acc.tile([128, 2, W2], F16, tag="mu16")
                nc.vector.tensor_copy(
                    out=mu16.rearrange("p a b -> p (a b)"), in_=mu32)
                rs16 = pb_acc.tile([128, 2, W2], F16, tag="rs16")
                nc.vector.tensor_copy(
                    out=rs16.rearrange("p a b -> p (a b)"), in_=rsL)
                with tc.tile_pool(name="pc_o", bufs=3) as pc_o:
                    for c in range(NCH):
                        ob = pc_o.tile([128, 2, W2], F16, tag="ob")
                        for mc in range(2):
                            tt = pc_o.tile([128, W2], F16, tag="tt")
                            nc.gpsimd.tensor_sub(tt, r_all[:, c, mc, :],
                                                 mu16[:, mc, :])
                            nc.vector.tensor_mul(tt, tt, rs16[:, mc, :])
                            nc.vector.tensor_scalar(
                                out=ob[:, mc, :], in0=tt,
                                scalar1=c_gamma[:, c:c + 1],
                                scalar2=c_beta[:, c:c + 1],
                                op0=OP.mult, op1=OP.add)
                        nc.sync.dma_start(
                            out[c].rearrange("(mc p) w -> p mc w", p=128), ob)
    nc.compile()
    return nc


_PROGRAM = None


def kernel(_trace=False, **inputs):
    global _PROGRAM
    np_in = {k: np.ascontiguousarray(np.asarray(v)) for k, v in inputs.items()}
    g, x = np_in["g"], np_in["x"]
    consts = build_consts()

    def pack_gx(a):
        p = np.zeros((C, H, W2), E4M3)
        p[:, :, :W] = a.astype(E4M3)
        return p

    def pack_w(wc, sl):
        return np.ascontiguousarray((wc[sl].T * CS).astype(E4M3))

    def pack_filt(f):
        # f [32, H, WF, 2] -> [16, 4(c,ri), H, WF] fp16, pre-divided by CS
        # to undo the x16 fp8 scaling of the dht constants.
        m = np.moveaxis(f, 3, 1) * (1.0 / CS)  # [32, 2, H, WF]
        return np.ascontiguousarray(
            m.reshape(16, 4, H, WF).astype(np.float16))

    in_maps = []
    for k in range(N_CORES):
        b, grp = k // 4, k % 4
        sl = slice(grp * NCH, (grp + 1) * NCH)
        xr = np.zeros((NCH, H, W2), np.float16)
        xr[:, :, :W] = x[b][sl].astype(np.float16)
        xr[:, 0, :W] += _corr_w().astype(np.float16)
        m = dict(
            g8=pack_gx(g[b]),
            x8=pack_gx(x[b]),
            wgT8=pack_w(np_in["wg_conv"], sl),
            wxT8=pack_w(np_in["wx_conv"], sl),
            bg=np.ascontiguousarray(
                np.tile(np_in["bg_conv"][sl], 4)[:, None]).astype(np.float32),
            bx=np.ascontiguousarray(
                np.tile(np_in["bx_conv"][sl], 4)[:, None]).astype(np.float32),
            fpg=pack_filt(np_in["filt_g"][sl]),
            fpx=pack_filt(np_in["filt_x"][sl]),
            xres=xr,
            gamma=np.ascontiguousarray(
                np_in["ln_gamma"][sl][None, :]).astype(np.float32),
            beta=np.ascontiguousarray(
                np_in["ln_beta"][sl][None, :]).astype(np.float32),
            **consts,
        )
        in_maps.append(m)
    if _PROGRAM is None:
        _PROGRAM = build_program()
    res = run_bass_kernel_spmd(_PROGRAM, in_maps, core_ids=list(range(N_CORES)),
                               trace=_trace)
    outf = np.zeros((B, C, H, W), np.float32)
    for k in range(N_CORES):
        b, grp = k // 4, k % 4
        outf[b, grp * NCH:(grp + 1) * NCH] = \
            res.results[k]["out"][:, :, :W].astype(np.float32)
    if _trace:
        kernel.last_results = res
    return outf


if __name__ == "__main__":
    ins = {
        "g": np.random.randn(B, C, H, W).astype(np.float32),
        "x": np.random.randn(B, C, H, W).astype(np.float32),
        "wg_conv": (np.random.randn(C, C) * 0.05).astype(np.float32),
        "bg_conv": np.zeros(C, np.float32),
        "wx_conv": (np.random.randn(C, C) * 0.05).astype(np.float32),
        "bx_conv": np.zeros(C, np.float32),
        "filt_g": (np.random.randn(C, H, WF, 2) * 0.02).astype(np.float32),
        "filt_x": (np.random.randn(C, H, WF, 2) * 0.02).astype(np.float32),
        "ln_gamma": np.ones(C, np.float32),
        "ln_beta": np.zeros(C, np.float32),
    }
    o = kernel(**ins)
    print("kernel ran, out shape", o.shape)
